# revision 11
# baseline (speedup 1.0000x reference)
"""BERT self-attention (B=16, T=512, C=768, H=12, D=64) on 8 trn2 NeuronCores.

Data-parallel over batch: each core gets 2 batches. Per core:
  qkv   = x @ W_attn + b_attn  computed as Q^T/K^T ([feature, token] layout,
          lhsT = W_attn tile) and V ([token, feature] layout, lhsT = x^T tile).
  S^T   = K^T-as-lhsT matmul -> scores in [key, query] layout (K=64, head pairs
          packed in PE row groups).
  P     = exp(S/8 + mask) on ScalarE (mask is a per-partition bias in this layout).
  y^T   = lhsT=[V_h | ones] matmul -> unnormalized y^T plus the softmax row-sums
          as an extra output row; normalize with DVE mul by a GPSIMD-broadcast
          reciprocal.
  out   = y^T-as-lhsT matmul with W_proj + b_proj, DMA'd PSUM->DRAM.
Biases are folded in as K=1 accumulating matmuls against a ones row.
"""

import sys

sys.path.insert(0, "/opt/trn_rl_repo")

from contextlib import ExitStack

import numpy as np

B, T, C = 16, 512, 768
H, D = 12, 64
C3 = 3 * C
N_CORES = 8
BC = B // N_CORES           # batches per core
M = BC * T                  # tokens per core
KT = C // 128               # feature k-tiles (6)
TT = M // 128               # token tiles per core (8)
NQK = 2 * C // 128          # q+k feature n-tiles (12)
VW = H * (D + 1)            # v tile width with interleaved ones cols (780)
SCALE = 1.0 / np.sqrt(D)

_cache = {}


def _build():
    import concourse.bass as bass
    import concourse.tile as tile
    from concourse import bacc, mybir
    from concourse.masks import make_identity

    f32 = mybir.dt.float32
    f32r = mybir.dt.float32r

    def r(ap):  # operands are stored as f32r already
        return ap

    nc = bacc.Bacc("TRN2", target_bir_lowering=False, debug=False,
                   num_devices=N_CORES)
    x_d = nc.dram_tensor("x", [M, C], f32, kind="ExternalInput").ap()
    mask_d = nc.dram_tensor("mask", [BC, T], f32, kind="ExternalInput").ap()
    wa_d = nc.dram_tensor("w_attn", [C, C3], f32, kind="ExternalInput").ap()
    ba_d = nc.dram_tensor("b_attn", [1, C3], f32, kind="ExternalInput").ap()
    wp_d = nc.dram_tensor("w_proj", [C, C], f32, kind="ExternalInput").ap()
    bp_d = nc.dram_tensor("b_proj", [1, C], f32, kind="ExternalInput").ap()
    out_d = nc.dram_tensor("out", [M, C], f32, kind="ExternalOutput").ap()

    Exp = mybir.ActivationFunctionType.Exp

    with tile.TileContext(nc) as tc, ExitStack() as ctx:
        persist = ctx.enter_context(tc.tile_pool(name="persist", bufs=1))
        qk_pool = ctx.enter_context(tc.tile_pool(name="qkt", bufs=1))

        ones_src = persist.tile([1, M], f32, tag="ones_src")
        nc.vector.memset(ones_src[:], 1.0)
        ones = persist.tile([1, M], f32r, tag="ones")
        nc.any.tensor_copy(ones[:], ones_src[:])
        onesP = persist.tile([128, H], f32, tag="onesP")
        nc.vector.memset(onesP[:], 1.0)
        mask_sb = persist.tile([128, BC * 4], f32, tag="mask")
        nc.sync.dma_start(
            mask_sb[:], mask_d.rearrange("a b -> (a b)").rearrange("(j p) -> p j", p=128)
        )
        ba_t = persist.tile([1, C3], f32r, tag="ba")
        nc.sync.dma_start(ba_t[:], ba_d[:].bitcast(f32r))
        bp_t = persist.tile([1, C], f32r, tag="bp")
        nc.sync.dma_start(bp_t[:], bp_d[:].bitcast(f32r))

        v_t = [persist.tile([128, VW], f32r, tag=f"v{t}", name=f"v{t}") for t in range(TT)]
        qkT = [qk_pool.tile([128, M], f32r, tag=f"qk{n}", name=f"qk{n}") for n in range(NQK)]

        with ExitStack() as load_ctx:
            ld = load_ctx.enter_context(tc.tile_pool(name="load", bufs=1))
            ps_tr = load_ctx.enter_context(
                tc.tile_pool(name="ps_tr", bufs=2, space="PSUM"))
            ps_mm = load_ctx.enter_context(
                tc.tile_pool(name="ps_mm", bufs=4, space="PSUM"))

            # ---- load x and W_attn, build xT via PE transposes ----
            ident = ld.tile([128, 128], f32, tag="ident")
            make_identity(nc, ident[:])
            wa_t = []
            for k in range(KT):
                wt = ld.tile([128, C3], f32r, tag=f"wa{k}")
                nc.sync.dma_start(wt[:], wa_d[k * 128:(k + 1) * 128, :].bitcast(f32r))
                wa_t.append(wt)
            xT = [ld.tile([128, M], f32r, tag=f"xT{k}", name=f"xT{k}") for k in range(KT)]
            for t in range(TT):
                xt = ld.tile([128, C], f32, tag="x_in", bufs=4)
                nc.sync.dma_start(xt[:], x_d[t * 128:(t + 1) * 128, :])
                for k in range(KT):
                    p = ps_tr.tile([128, 128], f32)
                    nc.tensor.transpose(
                        p[:], xt[:, k * 128:(k + 1) * 128], ident[:])
                    nc.any.tensor_copy(xT[k][:, t * 128:(t + 1) * 128], p[:])

            # ---- Q^T / K^T:  out[n_tile, tok] = W.T @ x.T ----
            for n in range(NQK):
                for mc in range(M // 512):
                    p = ps_mm.tile([128, 512], f32)
                    for k in range(KT):
                        nc.tensor.matmul(
                            p[:],
                            r(wa_t[k][:, n * 128:(n + 1) * 128]),
                            r(xT[k][:, mc * 512:(mc + 1) * 512]),
                            start=(k == 0), stop=False)
                    nc.tensor.matmul(
                        p[:],
                        r(ba_t[0:1, n * 128:(n + 1) * 128]),
                        r(ones[0:1, mc * 512:(mc + 1) * 512]),
                        start=False, stop=True)
                    nc.any.tensor_copy(qkT[n][:, mc * 512:(mc + 1) * 512], p[:])

            # ---- V: out[tok_tile, feat] = x @ W_v, heads strided by 65 ----
            for t in range(TT):
                for lo, w in ((0, 512), (512, 256)):
                    p = ps_mm.tile([128, 512], f32)
                    for k in range(KT):
                        nc.tensor.matmul(
                            p[:, :w],
                            r(xT[k][:, t * 128:(t + 1) * 128]),
                            r(wa_t[k][:, 2 * C + lo:2 * C + lo + w]),
                            start=(k == 0), stop=False)
                    nc.tensor.matmul(
                        p[:, :w],
                        r(ones[0:1, t * 128:(t + 1) * 128]),
                        r(ba_t[0:1, 2 * C + lo:2 * C + lo + w]),
                        start=False, stop=True)
                    h0 = lo // D
                    nc.any.tensor_copy(
                        v_t[t].rearrange("p (h c) -> p h c", c=D + 1)
                            [:, h0:h0 + w // D, 0:D],
                        p[:, :w].rearrange("p (h c) -> p h c", c=D))
                nc.any.tensor_copy(
                    v_t[t].rearrange("p (h c) -> p h c", c=D + 1)[:, :, D:D + 1],
                    onesP[:].rearrange("p (h o) -> p h o", o=1))

        yT_pool = ctx.enter_context(tc.tile_pool(name="yT", bufs=1))
        yT_t = [yT_pool.tile([128, M], f32r, tag=f"yT{c}", name=f"yT{c}")
                for c in range(KT)]

        # ---- attention per (batch, head) ----
        with ExitStack() as att_ctx:
            ap_ = att_ctx.enter_context(tc.tile_pool(name="att", bufs=3))
            np_ = att_ctx.enter_context(tc.tile_pool(name="norm", bufs=3))
            ps_s = att_ctx.enter_context(
                tc.tile_pool(name="ps_s", bufs=3, space="PSUM"))
            ps_y = att_ctx.enter_context(
                tc.tile_pool(name="ps_y", bufs=2, space="PSUM"))

            for b in range(BC):
                for h in range(H):
                    nt, r0 = h // 2, 64 * (h % 2)
                    bcol = b * T
                    e_tiles = []
                    for kt in range(4):
                        ps = ps_s.tile([128, 512], f32)
                        nc.tensor.matmul(
                            ps[:],
                            r(qkT[6 + nt][r0:r0 + D,
                                          bcol + kt * 128:bcol + (kt + 1) * 128]),
                            r(qkT[nt][r0:r0 + D, bcol:bcol + T]),
                            start=True, stop=True)
                        e = ap_.tile([128, 512], f32r, tag="e")
                        nc.scalar.activation(
                            e[:], ps[:], Exp,
                            bias=mask_sb[:, b * 4 + kt:b * 4 + kt + 1],
                            scale=float(SCALE))
                        e_tiles.append(e)
                    py = ps_y.tile([128, 512], f32)
                    for kt in range(4):
                        nc.tensor.matmul(
                            py[0:D + 1, :],
                            r(v_t[b * 4 + kt][:, (D + 1) * h:(D + 1) * (h + 1)]),
                            r(e_tiles[kt][:]),
                            start=(kt == 0), stop=(kt == 3))
                    recip = np_.tile([D + 1, 512], f32, tag="recip")
                    nc.vector.reciprocal(recip[D:D + 1, :], py[D:D + 1, :])
                    rep = np_.tile([64, 512], f32, tag="rep")
                    nc.sync.dma_start(
                        rep[:],
                        recip[D:D + 1, None, :].broadcast_to((1, 64, 512)))
                    dst = yT_t[nt][r0:r0 + D, bcol:bcol + T]
                    if r0 == 0:
                        nc.vector.tensor_mul(dst, py[0:D, :], rep[:])
                    else:
                        st = np_.tile([64, 512], f32r, tag="stage")
                        nc.vector.tensor_mul(st[:], py[0:D, :], rep[:])
                        nc.sync.dma_start(dst, st[:])

        # ---- projection: out[tok_tile, c'] = y @ W_proj + b_proj ----
        with ExitStack() as proj_ctx:
            pl = proj_ctx.enter_context(tc.tile_pool(name="projw", bufs=1))
            ps_o = proj_ctx.enter_context(
                tc.tile_pool(name="ps_o", bufs=3, space="PSUM"))
            ps_ob = proj_ctx.enter_context(tc.tile_pool(name="ps_ob", bufs=3))
            wp_t = []
            for k in range(KT):
                wt = pl.tile([128, C], f32r, tag=f"wp{k}")
                nc.sync.dma_start(wt[:], wp_d[k * 128:(k + 1) * 128, :].bitcast(f32r))
                wp_t.append(wt)
            for t in range(TT):
                for lo, w in ((0, 512), (512, 256)):
                    p = ps_o.tile([128, 512], f32)
                    for k in range(KT):
                        nc.tensor.matmul(
                            p[:, :w],
                            r(yT_t[k][:, t * 128:(t + 1) * 128]),
                            r(wp_t[k][:, lo:lo + w]),
                            start=(k == 0), stop=False)
                    nc.tensor.matmul(
                        p[:, :w],
                        r(ones[0:1, t * 128:(t + 1) * 128]),
                        r(bp_t[0:1, lo:lo + w]),
                        start=False, stop=True)
                    ot = ps_ob.tile([128, 512], f32, tag="ostage")
                    nc.any.tensor_copy(ot[:, :w], p[:, :w])
                    nc.sync.dma_start(
                        out_d[t * 128:(t + 1) * 128, lo:lo + w], ot[:, :w])

    nc.compile()
    return nc


def get_compiled():
    if "nc" not in _cache:
        _cache["nc"] = _build()
    return _cache["nc"]


def make_in_maps(x, attention_mask, W_attn, b_attn, W_proj, b_proj):
    x = np.ascontiguousarray(np.asarray(x, dtype=np.float32))
    mask = np.ascontiguousarray(
        np.asarray(attention_mask, dtype=np.float32)[:, 0, 0, :])
    wa = np.ascontiguousarray(np.asarray(W_attn, dtype=np.float32))
    ba = np.ascontiguousarray(np.asarray(b_attn, dtype=np.float32)).reshape(1, C3)
    wp = np.ascontiguousarray(np.asarray(W_proj, dtype=np.float32))
    bp = np.ascontiguousarray(np.asarray(b_proj, dtype=np.float32)).reshape(1, C)
    maps = []
    for i in range(N_CORES):
        maps.append({
            "x": np.ascontiguousarray(x[BC * i:BC * (i + 1)].reshape(M, C)),
            "mask": np.ascontiguousarray(mask[BC * i:BC * (i + 1)]),
            "w_attn": wa, "b_attn": ba, "w_proj": wp, "b_proj": bp,
        })
    return maps


def kernel(x, attention_mask, W_attn, b_attn, W_proj, b_proj):
    from concourse.bass_utils import run_bass_kernel_spmd

    nc = get_compiled()
    in_maps = make_in_maps(x, attention_mask, W_attn, b_attn, W_proj, b_proj)
    last_err = None
    for _ in range(3):
        try:
            res = run_bass_kernel_spmd(nc, in_maps, list(range(N_CORES)))
            break
        except Exception as e:  # transient NRT device errors: retry
            last_err = e
    else:
        raise last_err
    out = np.concatenate(
        [res.results[i]["out"].reshape(BC, T, C) for i in range(N_CORES)], axis=0)
    return out.astype(np.float32)


# revision 14
# speedup vs baseline: 1.0751x; 1.0751x over previous
"""BERT self-attention (B=16, T=512, C=768, H=12, D=64) on 8 trn2 NeuronCores.

Data-parallel over batch: each core gets 2 batches. Matmul operands are fp16
(11-bit mantissa, ~tf32-class precision, 1 cycle/row PE streaming, FWL weight
loads); all accumulation stays fp32 in PSUM. Per core:
  xT    = x transposed during load via the DMA XBAR transpose (fp16).
  Q^T/K^T ([feature, token] layout, lhsT = W_attn tile) and V ([token, feature]
          layout with an interleaved ones column per head, lhsT = xT tile).
  S^T   = K^T-as-lhsT matmul -> scores in [key, query] layout (K=64, head pairs
          packed in PE row groups via base-partition-64 slices).
  P     = exp(S/8 + mask) on ScalarE (mask is a per-partition bias in this
          layout), written as fp16.
  y^T   = lhsT=[V_h | ones] matmul -> unnormalized y^T plus softmax row-sums as
          an extra PSUM row; row-sums are collected per batch, inverted in one
          batched DVE reciprocal, replicated across partitions by a
          broadcast-AP DMA, and applied with a DVE multiply.
  out   = y^T-as-lhsT matmul with W_proj + b_proj (fp32 result to DRAM).
Biases are folded in as K=1 accumulating matmuls against a ones row.
"""

import sys

sys.path.insert(0, "/opt/trn_rl_repo")

from contextlib import ExitStack

import numpy as np

B, T, C = 16, 512, 768
H, D = 12, 64
C3 = 3 * C
N_CORES = 8
BC = B // N_CORES           # batches per core
M = BC * T                  # tokens per core
KT = C // 128               # feature k-tiles (6)
TT = M // 128               # token tiles per core (8)
NQK = 2 * C // 128          # q+k feature n-tiles (12)
VW = H * (D + 1)            # v tile width with interleaved ones cols (780)
SCALE = 1.0 / np.sqrt(D)

_cache = {}


def _build():
    import concourse.bass as bass
    import concourse.tile as tile
    from concourse import bacc, mybir
    f32 = mybir.dt.float32
    f16 = mybir.dt.float16
    Exp = mybir.ActivationFunctionType.Exp

    nc = bacc.Bacc("TRN2", target_bir_lowering=False, debug=False,
                   num_devices=N_CORES)
    x_d = nc.dram_tensor("x", [M, C], f16, kind="ExternalInput").ap()
    mask_d = nc.dram_tensor("mask", [BC, T], f32, kind="ExternalInput").ap()
    wa_d = nc.dram_tensor("w_attn", [C, C3], f16, kind="ExternalInput").ap()
    ba_d = nc.dram_tensor("b_attn", [1, C3], f16, kind="ExternalInput").ap()
    wp_d = nc.dram_tensor("w_proj", [C, C], f16, kind="ExternalInput").ap()
    bp_d = nc.dram_tensor("b_proj", [1, C], f16, kind="ExternalInput").ap()
    out_d = nc.dram_tensor("out", [M, C], f32, kind="ExternalOutput").ap()

    with tile.TileContext(nc) as tc, ExitStack() as ctx:
        persist = ctx.enter_context(tc.tile_pool(name="persist", bufs=1))
        qk_pool = ctx.enter_context(tc.tile_pool(name="qkt", bufs=1))

        ones = persist.tile([1, M], f16, tag="ones")
        nc.vector.memset(ones[:], 1.0)
        mask_sb = persist.tile([128, BC * 4], f32, tag="mask")
        nc.sync.dma_start(
            mask_sb[:],
            mask_d.rearrange("a b -> (a b)").rearrange("(j p) -> p j", p=128))
        ba_t = persist.tile([1, C3], f16, tag="ba")
        nc.sync.dma_start(ba_t[:], ba_d[:])
        bp_t = persist.tile([1, C], f16, tag="bp")
        nc.sync.dma_start(bp_t[:], bp_d[:])

        v_t = [persist.tile([128, VW], f16, tag=f"v{t}", name=f"v{t}")
               for t in range(TT)]
        qkT = [qk_pool.tile([128, M], f16, tag=f"qk{n}", name=f"qk{n}")
               for n in range(NQK)]

        with ExitStack() as load_ctx:
            ld = load_ctx.enter_context(tc.tile_pool(name="load", bufs=1))
            ps_mm = load_ctx.enter_context(
                tc.tile_pool(name="ps_mm", bufs=4, space="PSUM"))

            # ---- load W_attn; load x transposed via DMA XBAR ----
            wa_t = []
            for k in range(KT):
                wt = ld.tile([128, C3], f16, tag=f"wa{k}")
                nc.sync.dma_start(wt[:], wa_d[k * 128:(k + 1) * 128, :])
                wa_t.append(wt)
            xT = [ld.tile([128, M], f16, tag=f"xT{k}", name=f"xT{k}")
                  for k in range(KT)]
            for k in range(KT):
                nc.sync.dma_start(
                    xT[k][:], x_d[:, k * 128:(k + 1) * 128], transpose=True)

            # ---- Q^T / K^T:  out[n_tile, tok] = W.T @ x.T ----
            for n in range(NQK):
                for mc in range(M // 512):
                    p = ps_mm.tile([128, 512], f32)
                    for k in range(KT):
                        nc.tensor.matmul(
                            p[:],
                            wa_t[k][:, n * 128:(n + 1) * 128],
                            xT[k][:, mc * 512:(mc + 1) * 512],
                            start=(k == 0), stop=False)
                    nc.tensor.matmul(
                        p[:],
                        ba_t[0:1, n * 128:(n + 1) * 128],
                        ones[0:1, mc * 512:(mc + 1) * 512],
                        start=False, stop=True)
                    nc.any.tensor_copy(qkT[n][:, mc * 512:(mc + 1) * 512], p[:])

            # ---- V: out[tok_tile, feat] = x @ W_v, heads strided by 65 ----
            for t in range(TT):
                for lo, w in ((0, 512), (512, 256)):
                    p = ps_mm.tile([128, 512], f32)
                    for k in range(KT):
                        nc.tensor.matmul(
                            p[:, :w],
                            xT[k][:, t * 128:(t + 1) * 128],
                            wa_t[k][:, 2 * C + lo:2 * C + lo + w],
                            start=(k == 0), stop=False)
                    nc.tensor.matmul(
                        p[:, :w],
                        ones[0:1, t * 128:(t + 1) * 128],
                        ba_t[0:1, 2 * C + lo:2 * C + lo + w],
                        start=False, stop=True)
                    h0 = lo // D
                    nc.any.tensor_copy(
                        v_t[t].rearrange("p (h c) -> p h c", c=D + 1)
                            [:, h0:h0 + w // D, 0:D],
                        p[:, :w].rearrange("p (h c) -> p h c", c=D))
                nc.vector.memset(
                    v_t[t].rearrange("p (h c) -> p h c", c=D + 1)
                        [:, :, D:D + 1], 1.0)

        yT_pool = ctx.enter_context(tc.tile_pool(name="yT", bufs=1))
        yT_t = [yT_pool.tile([128, M], f16, tag=f"yT{c}", name=f"yT{c}")
                for c in range(KT)]

        # ---- attention per (batch, head) ----
        with ExitStack() as att_ctx:
            ap_ = att_ctx.enter_context(tc.tile_pool(name="att", bufs=6))
            np_ = att_ctx.enter_context(tc.tile_pool(name="norm", bufs=4))
            ps_s = att_ctx.enter_context(
                tc.tile_pool(name="ps_s", bufs=3, space="PSUM"))
            ps_y = att_ctx.enter_context(
                tc.tile_pool(name="ps_y", bufs=5, space="PSUM"))

            G = 4  # heads per reciprocal batch (bounded by PSUM banks)
            for b in range(BC):
                bcol = b * T
                for g in range(H // G):
                    py_tiles = []
                    r_all = np_.tile([G, 512], f32, tag="r_all")
                    for h in range(g * G, (g + 1) * G):
                        nt, r0 = h // 2, 64 * (h % 2)
                        e_tiles = []
                        for kt in range(4):
                            ps = ps_s.tile([128, 512], f32)
                            nc.tensor.matmul(
                                ps[:],
                                qkT[6 + nt][r0:r0 + D,
                                            bcol + kt * 128:
                                            bcol + (kt + 1) * 128],
                                qkT[nt][r0:r0 + D, bcol:bcol + T],
                                start=True, stop=True)
                            e = ap_.tile([128, 512], f16, tag="e")
                            nc.scalar.activation(
                                e[:], ps[:], Exp,
                                bias=mask_sb[:, b * 4 + kt:b * 4 + kt + 1],
                                scale=float(SCALE))
                            e_tiles.append(e)
                        py = ps_y.tile([128, 512], f32)
                        for kt in range(4):
                            nc.tensor.matmul(
                                py[0:D + 1, :],
                                v_t[b * 4 + kt][:, (D + 1) * h:
                                                (D + 1) * (h + 1)],
                                e_tiles[kt][:],
                                start=(kt == 0), stop=(kt == 3))
                        py_tiles.append(py)
                        # hop the PSUM r row into the group tile (engine ops
                        # cannot shift partitions; DMA cannot read PSUM)
                        rs = np_.tile([D + 1, 512], f32, tag="rstage")
                        nc.any.tensor_copy(rs[D:D + 1, :], py[D:D + 1, :])
                        nc.sync.dma_start(
                            r_all[h % G:h % G + 1, :], rs[D:D + 1, :])
                    recip = np_.tile([G, 512], f32, tag="recip")
                    nc.vector.reciprocal(recip[:], r_all[:])
                    for h in range(g * G, (g + 1) * G):
                        nt, r0 = h // 2, 64 * (h % 2)
                        py = py_tiles[h % G]
                        rep = np_.tile([64, 512], f32, tag="rep")
                        nc.sync.dma_start(
                            rep[:],
                            recip[h % G:h % G + 1, None, :]
                            .broadcast_to((1, 64, 512)))
                        dst = yT_t[nt][r0:r0 + D, bcol:bcol + T]
                        if r0 == 0:
                            nc.vector.tensor_mul(dst, py[0:D, :], rep[:])
                        else:
                            st = np_.tile([64, 512], f16, tag="stage")
                            nc.vector.tensor_mul(st[:], py[0:D, :], rep[:])
                            nc.sync.dma_start(dst, st[:])

        # ---- projection: out[tok_tile, c'] = y @ W_proj + b_proj ----
        with ExitStack() as proj_ctx:
            pl = proj_ctx.enter_context(tc.tile_pool(name="projw", bufs=1))
            ps_o = proj_ctx.enter_context(
                tc.tile_pool(name="ps_o", bufs=3, space="PSUM"))
            ps_ob = proj_ctx.enter_context(tc.tile_pool(name="ps_ob", bufs=3))
            wp_t = []
            for k in range(KT):
                wt = pl.tile([128, C], f16, tag=f"wp{k}")
                nc.sync.dma_start(wt[:], wp_d[k * 128:(k + 1) * 128, :])
                wp_t.append(wt)
            for t in range(TT):
                for lo, w in ((0, 512), (512, 256)):
                    p = ps_o.tile([128, 512], f32)
                    for k in range(KT):
                        nc.tensor.matmul(
                            p[:, :w],
                            yT_t[k][:, t * 128:(t + 1) * 128],
                            wp_t[k][:, lo:lo + w],
                            start=(k == 0), stop=False)
                    nc.tensor.matmul(
                        p[:, :w],
                        ones[0:1, t * 128:(t + 1) * 128],
                        bp_t[0:1, lo:lo + w],
                        start=False, stop=True)
                    ot = ps_ob.tile([128, 512], f32, tag="ostage")
                    nc.any.tensor_copy(ot[:, :w], p[:, :w])
                    nc.sync.dma_start(
                        out_d[t * 128:(t + 1) * 128, lo:lo + w], ot[:, :w])

    nc.compile()
    return nc


def get_compiled():
    if "nc" not in _cache:
        _cache["nc"] = _build()
    return _cache["nc"]


def make_in_maps(x, attention_mask, W_attn, b_attn, W_proj, b_proj):
    x = np.asarray(x, dtype=np.float32).astype(np.float16)
    mask = np.ascontiguousarray(
        np.asarray(attention_mask, dtype=np.float32)[:, 0, 0, :])
    wa = np.asarray(W_attn, dtype=np.float32).astype(np.float16)
    ba = np.asarray(b_attn, dtype=np.float32).astype(np.float16).reshape(1, C3)
    wp = np.asarray(W_proj, dtype=np.float32).astype(np.float16)
    bp = np.asarray(b_proj, dtype=np.float32).astype(np.float16).reshape(1, C)
    maps = []
    for i in range(N_CORES):
        maps.append({
            "x": np.ascontiguousarray(x[BC * i:BC * (i + 1)].reshape(M, C)),
            "mask": np.ascontiguousarray(mask[BC * i:BC * (i + 1)]),
            "w_attn": wa, "b_attn": ba, "w_proj": wp, "b_proj": bp,
        })
    return maps


def kernel(x, attention_mask, W_attn, b_attn, W_proj, b_proj):
    from concourse.bass_utils import run_bass_kernel_spmd

    nc = get_compiled()
    in_maps = make_in_maps(x, attention_mask, W_attn, b_attn, W_proj, b_proj)
    last_err = None
    for _ in range(3):
        try:
            res = run_bass_kernel_spmd(nc, in_maps, list(range(N_CORES)))
            break
        except Exception as e:  # transient NRT device errors: retry
            last_err = e
    else:
        raise last_err
    out = np.concatenate(
        [res.results[i]["out"].reshape(BC, T, C) for i in range(N_CORES)], axis=0)
    return out.astype(np.float32)


# revision 15
# speedup vs baseline: 1.1974x; 1.1137x over previous
"""BERT self-attention (B=16, T=512, C=768, H=12, D=64) on 8 trn2 NeuronCores.

Data-parallel over batch: each core gets 2 batches. Matmul operands are fp16
(11-bit mantissa, ~tf32-class precision, 1 cycle/row PE streaming, FWL weight
loads); all accumulation stays fp32 in PSUM. Per core:
  xT    = x transposed during load via the DMA XBAR transpose (fp16).
  Q^T/K^T ([feature, token] layout, lhsT = W_attn tile) and V ([token, feature]
          layout with an interleaved ones column per head, lhsT = xT tile).
  S^T   = K^T-as-lhsT matmul -> scores in [key, query] layout (K=64, head pairs
          packed in PE row groups via base-partition-64 slices).
  P     = exp(S/8 + mask) on ScalarE (mask is a per-partition bias in this
          layout), written as fp16.
  y^T   = lhsT=[V_h | ones] matmul -> unnormalized y^T plus softmax row-sums as
          an extra PSUM row; row-sums are collected per batch, inverted in one
          batched DVE reciprocal, replicated across partitions by a
          broadcast-AP DMA, and applied with a DVE multiply.
  out   = y^T-as-lhsT matmul with W_proj + b_proj (fp32 result to DRAM).
Biases are folded in as K=1 accumulating matmuls against a ones row.
"""

import sys

sys.path.insert(0, "/opt/trn_rl_repo")

from contextlib import ExitStack

import numpy as np

B, T, C = 16, 512, 768
H, D = 12, 64
C3 = 3 * C
N_CORES = 8
BC = B // N_CORES           # batches per core
M = BC * T                  # tokens per core
KT = C // 128               # feature k-tiles (6)
TT = M // 128               # token tiles per core (8)
NQK = 2 * C // 128          # q+k feature n-tiles (12)
VW = H * (D + 1)            # v tile width with interleaved ones cols (780)
SCALE = 1.0 / np.sqrt(D)

_cache = {}


def _build():
    import concourse.bass as bass
    import concourse.tile as tile
    from concourse import bacc, mybir
    f32 = mybir.dt.float32
    f16 = mybir.dt.float16
    Exp = mybir.ActivationFunctionType.Exp

    nc = bacc.Bacc("TRN2", target_bir_lowering=False, debug=False,
                   num_devices=N_CORES)
    x_d = nc.dram_tensor("x", [M, C], f16, kind="ExternalInput").ap()
    mask_d = nc.dram_tensor("mask", [BC, T], f32, kind="ExternalInput").ap()
    wa_d = nc.dram_tensor("w_attn", [C, C3], f16, kind="ExternalInput").ap()
    ba_d = nc.dram_tensor("b_attn", [1, C3], f16, kind="ExternalInput").ap()
    wp_d = nc.dram_tensor("w_proj", [C, C], f16, kind="ExternalInput").ap()
    bp_d = nc.dram_tensor("b_proj", [1, C], f16, kind="ExternalInput").ap()
    out_d = nc.dram_tensor("out", [M, C], f32, kind="ExternalOutput").ap()

    with tile.TileContext(nc) as tc, ExitStack() as ctx:
        persist = ctx.enter_context(tc.tile_pool(name="persist", bufs=1))
        qk_pool = ctx.enter_context(tc.tile_pool(name="qkt", bufs=1))

        ones = persist.tile([1, M], f16, tag="ones")
        nc.vector.memset(ones[:], 1.0)
        mask_sb = persist.tile([128, BC * 4], f32, tag="mask")
        nc.sync.dma_start(
            mask_sb[:],
            mask_d.rearrange("a b -> (a b)").rearrange("(j p) -> p j", p=128))
        ba_t = persist.tile([1, C3], f16, tag="ba")
        nc.sync.dma_start(ba_t[:], ba_d[:])
        bp_t = persist.tile([1, C], f16, tag="bp")
        nc.sync.dma_start(bp_t[:], bp_d[:])

        v_t = [persist.tile([128, VW], f16, tag=f"v{t}", name=f"v{t}")
               for t in range(TT)]
        qkT = [qk_pool.tile([128, M], f16, tag=f"qk{n}", name=f"qk{n}")
               for n in range(NQK)]

        with ExitStack() as load_ctx:
            ld = load_ctx.enter_context(tc.tile_pool(name="load", bufs=1))
            ps_mm = load_ctx.enter_context(
                tc.tile_pool(name="ps_mm", bufs=4, space="PSUM"))

            # ---- load W_attn; load x transposed via DMA XBAR ----
            wa_t = []
            for k in range(KT):
                wt = ld.tile([128, C3], f16, tag=f"wa{k}")
                nc.sync.dma_start(wt[:], wa_d[k * 128:(k + 1) * 128, :])
                wa_t.append(wt)
            xT = [ld.tile([128, M], f16, tag=f"xT{k}", name=f"xT{k}")
                  for k in range(KT)]
            for k in range(KT):
                nc.sync.dma_start(
                    xT[k][:], x_d[:, k * 128:(k + 1) * 128], transpose=True)

            # ---- Q^T / K^T:  out[n_tile, tok] = W.T @ x.T ----
            for n in range(NQK):
                for mc in range(M // 512):
                    p = ps_mm.tile([128, 512], f32)
                    for k in range(KT):
                        nc.tensor.matmul(
                            p[:],
                            wa_t[k][:, n * 128:(n + 1) * 128],
                            xT[k][:, mc * 512:(mc + 1) * 512],
                            start=(k == 0), stop=False)
                    nc.tensor.matmul(
                        p[:],
                        ba_t[0:1, n * 128:(n + 1) * 128],
                        ones[0:1, mc * 512:(mc + 1) * 512],
                        start=False, stop=True)
                    nc.any.tensor_copy(qkT[n][:, mc * 512:(mc + 1) * 512], p[:])

            # ---- V: out[tok_tile, feat] = x @ W_v, heads strided by 65 ----
            for t in range(TT):
                for lo, w in ((0, 512), (512, 256)):
                    p = ps_mm.tile([128, 512], f32)
                    for k in range(KT):
                        nc.tensor.matmul(
                            p[:, :w],
                            xT[k][:, t * 128:(t + 1) * 128],
                            wa_t[k][:, 2 * C + lo:2 * C + lo + w],
                            start=(k == 0), stop=False)
                    nc.tensor.matmul(
                        p[:, :w],
                        ones[0:1, t * 128:(t + 1) * 128],
                        ba_t[0:1, 2 * C + lo:2 * C + lo + w],
                        start=False, stop=True)
                    h0 = lo // D
                    nc.any.tensor_copy(
                        v_t[t].rearrange("p (h c) -> p h c", c=D + 1)
                            [:, h0:h0 + w // D, 0:D],
                        p[:, :w].rearrange("p (h c) -> p h c", c=D))
                nc.vector.memset(
                    v_t[t].rearrange("p (h c) -> p h c", c=D + 1)
                        [:, :, D:D + 1], 1.0)

        yT_pool = ctx.enter_context(tc.tile_pool(name="yT", bufs=1))
        yT_t = [yT_pool.tile([128, M], f16, tag=f"yT{c}", name=f"yT{c}")
                for c in range(KT)]

        # ---- attention per (batch, head) ----
        with ExitStack() as att_ctx:
            ap_ = att_ctx.enter_context(tc.tile_pool(name="att", bufs=6))
            np_ = att_ctx.enter_context(tc.tile_pool(name="norm", bufs=4))
            ps_s = att_ctx.enter_context(
                tc.tile_pool(name="ps_s", bufs=3, space="PSUM"))
            ps_y = att_ctx.enter_context(
                tc.tile_pool(name="ps_y", bufs=2, space="PSUM"))

            for b in range(BC):
                bcol = b * T
                yun_tiles = []
                r_all = np_.tile([H, 512], f32, tag="r_all")
                for h in range(H):
                    nt, r0 = h // 2, 64 * (h % 2)
                    e_tiles = []
                    for kt in range(4):
                        ps = ps_s.tile([128, 512], f32)
                        nc.tensor.matmul(
                            ps[:],
                            qkT[6 + nt][r0:r0 + D,
                                        bcol + kt * 128:bcol + (kt + 1) * 128],
                            qkT[nt][r0:r0 + D, bcol:bcol + T],
                            start=True, stop=True)
                        e = ap_.tile([128, 512], f16, tag="e")
                        nc.scalar.activation(
                            e[:], ps[:], Exp,
                            bias=mask_sb[:, b * 4 + kt:b * 4 + kt + 1],
                            scale=float(SCALE))
                        e_tiles.append(e)
                    py = ps_y.tile([128, 512], f32)
                    for kt in range(4):
                        nc.tensor.matmul(
                            py[0:D + 1, :],
                            v_t[b * 4 + kt][:, (D + 1) * h:(D + 1) * (h + 1)],
                            e_tiles[kt][:],
                            start=(kt == 0), stop=(kt == 3))
                    # drain PSUM fast: unnormalized y^T to fp16 SBUF, r row
                    # staged fp32 then DMA-hopped into the batch r tile
                    # (engine ops cannot shift partitions; DMA can't read PSUM)
                    yun = np_.tile([64, 512], f16, tag="yun", bufs=14)
                    nc.any.tensor_copy(yun[:], py[0:D, :])
                    rs = np_.tile([D + 1, 512], f32, tag="rstage")
                    nc.any.tensor_copy(rs[D:D + 1, :], py[D:D + 1, :])
                    nc.sync.dma_start(r_all[h:h + 1, :], rs[D:D + 1, :])
                    yun_tiles.append(yun)
                # one reciprocal + one fp16 downconvert for all 12 heads
                recip = np_.tile([H, 512], f32, tag="recip")
                nc.vector.reciprocal(recip[:], r_all[:])
                recip16 = np_.tile([H, 512], f16, tag="recip16")
                nc.any.tensor_copy(recip16[:], recip[:])
                for h in range(H):
                    nt, r0 = h // 2, 64 * (h % 2)
                    rep = np_.tile([64, 512], f16, tag="rep")
                    nc.scalar.dma_start(
                        rep[:],
                        recip16[h:h + 1, None, :].broadcast_to((1, 64, 512)))
                    dst = yT_t[nt][r0:r0 + D, bcol:bcol + T]
                    if r0 == 0:
                        nc.vector.tensor_mul(dst, yun_tiles[h][:], rep[:])
                    else:
                        st = np_.tile([64, 512], f16, tag="stage")
                        nc.vector.tensor_mul(st[:], yun_tiles[h][:], rep[:])
                        nc.scalar.dma_start(dst, st[:])

        # ---- projection: out[tok_tile, c'] = y @ W_proj + b_proj ----
        with ExitStack() as proj_ctx:
            pl = proj_ctx.enter_context(tc.tile_pool(name="projw", bufs=1))
            ps_o = proj_ctx.enter_context(
                tc.tile_pool(name="ps_o", bufs=3, space="PSUM"))
            ps_ob = proj_ctx.enter_context(tc.tile_pool(name="ps_ob", bufs=3))
            wp_t = []
            for k in range(KT):
                wt = pl.tile([128, C], f16, tag=f"wp{k}")
                nc.sync.dma_start(wt[:], wp_d[k * 128:(k + 1) * 128, :])
                wp_t.append(wt)
            for t in range(TT):
                for lo, w in ((0, 512), (512, 256)):
                    p = ps_o.tile([128, 512], f32)
                    for k in range(KT):
                        nc.tensor.matmul(
                            p[:, :w],
                            yT_t[k][:, t * 128:(t + 1) * 128],
                            wp_t[k][:, lo:lo + w],
                            start=(k == 0), stop=False)
                    nc.tensor.matmul(
                        p[:, :w],
                        ones[0:1, t * 128:(t + 1) * 128],
                        bp_t[0:1, lo:lo + w],
                        start=False, stop=True)
                    ot = ps_ob.tile([128, 512], f32, tag="ostage")
                    nc.any.tensor_copy(ot[:, :w], p[:, :w])
                    nc.sync.dma_start(
                        out_d[t * 128:(t + 1) * 128, lo:lo + w], ot[:, :w])

    nc.compile()
    return nc


def get_compiled():
    if "nc" not in _cache:
        _cache["nc"] = _build()
    return _cache["nc"]


def make_in_maps(x, attention_mask, W_attn, b_attn, W_proj, b_proj):
    x = np.asarray(x, dtype=np.float32).astype(np.float16)
    mask = np.ascontiguousarray(
        np.asarray(attention_mask, dtype=np.float32)[:, 0, 0, :])
    wa = np.asarray(W_attn, dtype=np.float32).astype(np.float16)
    ba = np.asarray(b_attn, dtype=np.float32).astype(np.float16).reshape(1, C3)
    wp = np.asarray(W_proj, dtype=np.float32).astype(np.float16)
    bp = np.asarray(b_proj, dtype=np.float32).astype(np.float16).reshape(1, C)
    maps = []
    for i in range(N_CORES):
        maps.append({
            "x": np.ascontiguousarray(x[BC * i:BC * (i + 1)].reshape(M, C)),
            "mask": np.ascontiguousarray(mask[BC * i:BC * (i + 1)]),
            "w_attn": wa, "b_attn": ba, "w_proj": wp, "b_proj": bp,
        })
    return maps


def kernel(x, attention_mask, W_attn, b_attn, W_proj, b_proj):
    from concourse.bass_utils import run_bass_kernel_spmd

    nc = get_compiled()
    in_maps = make_in_maps(x, attention_mask, W_attn, b_attn, W_proj, b_proj)
    last_err = None
    for _ in range(3):
        try:
            res = run_bass_kernel_spmd(nc, in_maps, list(range(N_CORES)))
            break
        except Exception as e:  # transient NRT device errors: retry
            last_err = e
    else:
        raise last_err
    out = np.concatenate(
        [res.results[i]["out"].reshape(BC, T, C) for i in range(N_CORES)], axis=0)
    return out.astype(np.float32)


# revision 16
# speedup vs baseline: 1.2834x; 1.0719x over previous
"""BERT self-attention (B=16, T=512, C=768, H=12, D=64) on 8 trn2 NeuronCores.

Data-parallel over batch: each core gets 2 batches. Matmul operands are fp16
(11-bit mantissa, ~tf32-class precision, 1 cycle/row PE streaming, FWL weight
loads); all accumulation stays fp32 in PSUM. Per core:
  xT    = x transposed during load via the DMA XBAR transpose (fp16).
  Q^T/K^T ([feature, token] layout, lhsT = W_attn tile) and V ([token, feature]
          layout with an interleaved ones column per head, lhsT = xT tile).
  S^T   = K^T-as-lhsT matmul -> scores in [key, query] layout (K=64, head pairs
          packed in PE row groups via base-partition-64 slices).
  P     = exp(S/8 + mask) on ScalarE (mask is a per-partition bias in this
          layout), written as fp16.
  y^T   = lhsT=[V_h | ones] matmul -> unnormalized y^T plus softmax row-sums as
          an extra PSUM row; row-sums are collected per batch, inverted in one
          batched DVE reciprocal, replicated across partitions by a
          broadcast-AP DMA, and applied with a DVE multiply.
  out   = y^T-as-lhsT matmul with W_proj + b_proj (fp32 result to DRAM).
Biases are folded in as K=1 accumulating matmuls against a ones row.
"""

import sys

sys.path.insert(0, "/opt/trn_rl_repo")

from contextlib import ExitStack

import numpy as np

B, T, C = 16, 512, 768
H, D = 12, 64
C3 = 3 * C
N_CORES = 8
BC = B // N_CORES           # batches per core
M = BC * T                  # tokens per core
KT = C // 128               # feature k-tiles (6)
TT = M // 128               # token tiles per core (8)
NQK = 2 * C // 128          # q+k feature n-tiles (12)
VW = H * (D + 1)            # v tile width with interleaved ones cols (780)
SCALE = 1.0 / np.sqrt(D)

_cache = {}


def _build():
    import concourse.bass as bass
    import concourse.tile as tile
    from concourse import bacc, mybir
    f32 = mybir.dt.float32
    f16 = mybir.dt.float16
    Exp = mybir.ActivationFunctionType.Exp

    nc = bacc.Bacc("TRN2", target_bir_lowering=False, debug=False,
                   num_devices=N_CORES)
    x_d = nc.dram_tensor("x", [M, C], f16, kind="ExternalInput").ap()
    mask_d = nc.dram_tensor("mask", [BC, T], f32, kind="ExternalInput").ap()
    wa_d = nc.dram_tensor("w_attn", [C, C3], f16, kind="ExternalInput").ap()
    ba_d = nc.dram_tensor("b_attn", [1, C3], f16, kind="ExternalInput").ap()
    wp_d = nc.dram_tensor("w_proj", [C, C], f16, kind="ExternalInput").ap()
    bp_d = nc.dram_tensor("b_proj", [1, C], f16, kind="ExternalInput").ap()
    out_d = nc.dram_tensor("out", [M, C], f32, kind="ExternalOutput").ap()

    with tile.TileContext(nc) as tc, ExitStack() as ctx:
        pp = ctx.enter_context(tc.tile_pool(name="pp", bufs=1))
        np_ = ctx.enter_context(tc.tile_pool(name="norm", bufs=4))
        ap_ = ctx.enter_context(tc.tile_pool(name="att", bufs=6))
        ps_mm = ctx.enter_context(tc.tile_pool(name="ps_mm", bufs=2, space="PSUM"))
        ps_s = ctx.enter_context(tc.tile_pool(name="ps_s", bufs=2, space="PSUM"))
        ps_y = ctx.enter_context(tc.tile_pool(name="ps_y", bufs=2, space="PSUM"))

        ones = pp.tile([1, M], f16, tag="ones")
        nc.vector.memset(ones[:], 1.0)
        mask_sb = pp.tile([128, BC * 4], f32, tag="mask")
        nc.sync.dma_start(
            mask_sb[:],
            mask_d.rearrange("a b -> (a b)").rearrange("(j p) -> p j", p=128))
        ba_t = pp.tile([1, C3], f16, tag="ba")
        nc.sync.dma_start(ba_t[:], ba_d[:])
        bp_t = pp.tile([1, C], f16, tag="bp")
        nc.sync.dma_start(bp_t[:], bp_d[:])

        wa_t = []
        for k in range(KT):
            wt = pp.tile([128, C3], f16, tag=f"wa{k}", name=f"wa{k}")
            nc.sync.dma_start(wt[:], wa_d[k * 128:(k + 1) * 128, :])
            wa_t.append(wt)
        xT = [pp.tile([128, M], f16, tag=f"xT{k}", name=f"xT{k}")
              for k in range(KT)]
        for k in range(KT):
            nc.sync.dma_start(
                xT[k][:], x_d[:, k * 128:(k + 1) * 128], transpose=True)

        v_t = [pp.tile([128, VW], f16, tag=f"v{t}", name=f"v{t}")
               for t in range(TT)]
        qkT = [pp.tile([128, M], f16, tag=f"qk{n}", name=f"qk{n}")
               for n in range(NQK)]
        yT_t = [pp.tile([128, M], f16, tag=f"yT{c}", name=f"yT{c}")
                for c in range(KT)]
        wp_t = [pp.tile([128, C], f16, tag=f"wp{k}", name=f"wp{k}")
                for k in range(KT)]

        for b in range(BC):
            bcol = b * T

            # ---- Q^T/K^T for this batch: out[n_tile, tok] = W.T @ x.T ----
            for n in range(NQK):
                p = ps_mm.tile([128, 512], f32, tag="mm")
                for k in range(KT):
                    nc.tensor.matmul(
                        p[:],
                        wa_t[k][:, n * 128:(n + 1) * 128],
                        xT[k][:, bcol:bcol + T],
                        start=(k == 0), stop=False)
                nc.tensor.matmul(
                    p[:],
                    ba_t[0:1, n * 128:(n + 1) * 128],
                    ones[0:1, bcol:bcol + T],
                    start=False, stop=True)
                nc.vector.tensor_copy(qkT[n][:, bcol:bcol + T], p[:])

            # ---- V for this batch, heads strided by 65, ones col folded ----
            for t in range(b * 4, b * 4 + 4):
                for lo, w in ((0, 512), (512, 256)):
                    p = ps_mm.tile([128, 512], f32, tag="mm")
                    for k in range(KT):
                        nc.tensor.matmul(
                            p[:, :w],
                            xT[k][:, t * 128:(t + 1) * 128],
                            wa_t[k][:, 2 * C + lo:2 * C + lo + w],
                            start=(k == 0), stop=False)
                    nc.tensor.matmul(
                        p[:, :w],
                        ones[0:1, t * 128:(t + 1) * 128],
                        ba_t[0:1, 2 * C + lo:2 * C + lo + w],
                        start=False, stop=True)
                    h0 = lo // D
                    nc.vector.tensor_copy(
                        v_t[t].rearrange("p (h c) -> p h c", c=D + 1)
                            [:, h0:h0 + w // D, 0:D],
                        p[:, :w].rearrange("p (h c) -> p h c", c=D))
                nc.vector.memset(
                    v_t[t].rearrange("p (h c) -> p h c", c=D + 1)
                        [:, :, D:D + 1], 1.0)

            # ---- attention: head pairs share PSUM/exp tiles ----
            yun_tiles = []
            r_all = np_.tile([H, 512], f32, tag="r_all")
            for hp in range(H // 2):
                e_tiles = []
                for kt in range(4):
                    ps = ps_s.tile([128, 1024], f32)
                    for sub in range(2):
                        r0 = 64 * sub
                        nc.tensor.matmul(
                            ps[:, sub * 512:sub * 512 + 512],
                            qkT[6 + hp][r0:r0 + D,
                                        bcol + kt * 128:bcol + (kt + 1) * 128],
                            qkT[hp][r0:r0 + D, bcol:bcol + T],
                            start=True, stop=True)
                    e = ap_.tile([128, 1024], f16, tag="e")
                    nc.scalar.activation(
                        e[:], ps[:], Exp,
                        bias=mask_sb[:, b * 4 + kt:b * 4 + kt + 1],
                        scale=float(SCALE))
                    e_tiles.append(e)
                for sub in range(2):
                    h = 2 * hp + sub
                    py = ps_y.tile([128, 512], f32)
                    for kt in range(4):
                        nc.tensor.matmul(
                            py[0:D + 1, :],
                            v_t[b * 4 + kt][:, (D + 1) * h:(D + 1) * (h + 1)],
                            e_tiles[kt][:, sub * 512:sub * 512 + 512],
                            start=(kt == 0), stop=(kt == 3))
                    yun = np_.tile([64, 512], f16, tag="yun", bufs=14)
                    nc.vector.tensor_copy(yun[:], py[0:D, :])
                    rs = np_.tile([D + 1, 512], f32, tag="rstage")
                    nc.scalar.copy(rs[D:D + 1, :], py[D:D + 1, :])
                    nc.sync.dma_start(r_all[h:h + 1, :], rs[D:D + 1, :])
                    yun_tiles.append(yun)
            # one reciprocal + one fp16 downconvert for all 12 heads
            recip = np_.tile([H, 512], f32, tag="recip")
            nc.vector.reciprocal(recip[:], r_all[:])
            recip16 = np_.tile([H, 512], f16, tag="recip16")
            nc.vector.tensor_copy(recip16[:], recip[:])
            for h in range(H):
                nt, r0 = h // 2, 64 * (h % 2)
                rep = np_.tile([64, 512], f16, tag="rep")
                nc.sync.dma_start(
                    rep[:],
                    recip16[h:h + 1, None, :].broadcast_to((1, 64, 512)))
                dst = yT_t[nt][r0:r0 + D, bcol:bcol + T]
                if r0 == 0:
                    nc.vector.tensor_mul(dst, yun_tiles[h][:], rep[:])
                else:
                    st = np_.tile([64, 512], f16, tag="stage")
                    nc.vector.tensor_mul(st[:], yun_tiles[h][:], rep[:])
                    nc.sync.dma_start(dst, st[:])

            # ---- projection for this batch ----
            if b == 0:
                for k in range(KT):
                    nc.sync.dma_start(
                        wp_t[k][:], wp_d[k * 128:(k + 1) * 128, :])
            for t in range(b * 4, b * 4 + 4):
                for lo, w in ((0, 512), (512, 256)):
                    p = ps_mm.tile([128, 512], f32, tag="mm")
                    for k in range(KT):
                        nc.tensor.matmul(
                            p[:, :w],
                            yT_t[k][:, t * 128:(t + 1) * 128],
                            wp_t[k][:, lo:lo + w],
                            start=(k == 0), stop=False)
                    nc.tensor.matmul(
                        p[:, :w],
                        ones[0:1, t * 128:(t + 1) * 128],
                        bp_t[0:1, lo:lo + w],
                        start=False, stop=True)
                    ot = np_.tile([128, 512], f32, tag="ostage", bufs=3)
                    nc.vector.tensor_copy(ot[:, :w], p[:, :w])
                    nc.sync.dma_start(
                        out_d[t * 128:(t + 1) * 128, lo:lo + w], ot[:, :w])

    nc.compile()
    return nc


def get_compiled():
    if "nc" not in _cache:
        _cache["nc"] = _build()
    return _cache["nc"]


def make_in_maps(x, attention_mask, W_attn, b_attn, W_proj, b_proj):
    x = np.asarray(x, dtype=np.float32).astype(np.float16)
    mask = np.ascontiguousarray(
        np.asarray(attention_mask, dtype=np.float32)[:, 0, 0, :])
    wa = np.asarray(W_attn, dtype=np.float32).astype(np.float16)
    ba = np.asarray(b_attn, dtype=np.float32).astype(np.float16).reshape(1, C3)
    wp = np.asarray(W_proj, dtype=np.float32).astype(np.float16)
    bp = np.asarray(b_proj, dtype=np.float32).astype(np.float16).reshape(1, C)
    maps = []
    for i in range(N_CORES):
        maps.append({
            "x": np.ascontiguousarray(x[BC * i:BC * (i + 1)].reshape(M, C)),
            "mask": np.ascontiguousarray(mask[BC * i:BC * (i + 1)]),
            "w_attn": wa, "b_attn": ba, "w_proj": wp, "b_proj": bp,
        })
    return maps


def kernel(x, attention_mask, W_attn, b_attn, W_proj, b_proj):
    from concourse.bass_utils import run_bass_kernel_spmd

    nc = get_compiled()
    in_maps = make_in_maps(x, attention_mask, W_attn, b_attn, W_proj, b_proj)
    last_err = None
    for _ in range(3):
        try:
            res = run_bass_kernel_spmd(nc, in_maps, list(range(N_CORES)))
            break
        except Exception as e:  # transient NRT device errors: retry
            last_err = e
    else:
        raise last_err
    out = np.concatenate(
        [res.results[i]["out"].reshape(BC, T, C) for i in range(N_CORES)], axis=0)
    return out.astype(np.float32)


# revision 17
# speedup vs baseline: 1.3061x; 1.0176x over previous
"""BERT self-attention (B=16, T=512, C=768, H=12, D=64) on 8 trn2 NeuronCores.

Data-parallel over batch: each core gets 2 batches. Matmul operands are fp16
(11-bit mantissa, ~tf32-class precision, 1 cycle/row PE streaming, FWL weight
loads); all accumulation stays fp32 in PSUM. Per core:
  xT    = x transposed during load via the DMA XBAR transpose (fp16).
  Q^T/K^T ([feature, token] layout, lhsT = W_attn tile) and V ([token, feature]
          layout with an interleaved ones column per head, lhsT = xT tile).
  S^T   = K^T-as-lhsT matmul -> scores in [key, query] layout (K=64, head pairs
          packed in PE row groups via base-partition-64 slices).
  P     = exp(S/8 + mask) on ScalarE (mask is a per-partition bias in this
          layout), written as fp16.
  y^T   = lhsT=[V_h | ones] matmul -> unnormalized y^T plus softmax row-sums as
          an extra PSUM row; row-sums are collected per batch, inverted in one
          batched DVE reciprocal, replicated across partitions by a
          broadcast-AP DMA, and applied with a DVE multiply.
  out   = y^T-as-lhsT matmul with W_proj + b_proj (fp32 result to DRAM).
Biases are folded in as K=1 accumulating matmuls against a ones row.
"""

import sys

sys.path.insert(0, "/opt/trn_rl_repo")

from contextlib import ExitStack

import numpy as np

B, T, C = 16, 512, 768
H, D = 12, 64
C3 = 3 * C
N_CORES = 8
BC = B // N_CORES           # batches per core
M = BC * T                  # tokens per core
KT = C // 128               # feature k-tiles (6)
TT = M // 128               # token tiles per core (8)
NQK = 2 * C // 128          # q+k feature n-tiles (12)
VW = H * (D + 1)            # v tile width with interleaved ones cols (780)
SCALE = 1.0 / np.sqrt(D)

_cache = {}


def _build():
    import concourse.bass as bass
    import concourse.tile as tile
    from concourse import bacc, mybir
    f32 = mybir.dt.float32
    f16 = mybir.dt.float16
    Exp = mybir.ActivationFunctionType.Exp

    nc = bacc.Bacc("TRN2", target_bir_lowering=False, debug=False,
                   num_devices=N_CORES)
    x_d = nc.dram_tensor("x", [M, C], f16, kind="ExternalInput").ap()
    mask_d = nc.dram_tensor("mask", [BC, T], f32, kind="ExternalInput").ap()
    wa_d = nc.dram_tensor("w_attn", [C, C3], f16, kind="ExternalInput").ap()
    ba_d = nc.dram_tensor("b_attn", [1, C3], f16, kind="ExternalInput").ap()
    wp_d = nc.dram_tensor("w_proj", [C, C], f16, kind="ExternalInput").ap()
    bp_d = nc.dram_tensor("b_proj", [1, C], f16, kind="ExternalInput").ap()
    out_d = nc.dram_tensor("out", [M, C], f32, kind="ExternalOutput").ap()

    with tile.TileContext(nc) as tc, ExitStack() as ctx:
        pp = ctx.enter_context(tc.tile_pool(name="pp", bufs=1))
        np_ = ctx.enter_context(tc.tile_pool(name="norm", bufs=4))
        ap_ = ctx.enter_context(tc.tile_pool(name="att", bufs=6))
        ps_mm = ctx.enter_context(tc.tile_pool(name="ps_mm", bufs=2, space="PSUM"))
        ps_s = ctx.enter_context(tc.tile_pool(name="ps_s", bufs=2, space="PSUM"))
        ps_y = ctx.enter_context(tc.tile_pool(name="ps_y", bufs=2, space="PSUM"))

        ones = pp.tile([1, M], f16, tag="ones")
        nc.vector.memset(ones[:], 1.0)
        mask_sb = pp.tile([128, BC * 4], f32, tag="mask")
        nc.sync.dma_start(
            mask_sb[:],
            mask_d.rearrange("a b -> (a b)").rearrange("(j p) -> p j", p=128))
        ba_t = pp.tile([1, C3], f16, tag="ba")
        nc.sync.dma_start(ba_t[:], ba_d[:])
        bp_t = pp.tile([1, C], f16, tag="bp")
        nc.sync.dma_start(bp_t[:], bp_d[:])

        wa_t = [pp.tile([128, C3], f16, tag=f"wa{k}", name=f"wa{k}")
                for k in range(KT)]
        xT = [pp.tile([128, M], f16, tag=f"xT{k}", name=f"xT{k}")
              for k in range(KT)]
        for k in range(KT):
            nc.sync.dma_start(
                xT[k][:], x_d[:, k * 128:(k + 1) * 128], transpose=True)
            nc.sync.dma_start(wa_t[k][:], wa_d[k * 128:(k + 1) * 128, :])
        wp_t = [pp.tile([128, C], f16, tag=f"wp{k}", name=f"wp{k}")
                for k in range(KT)]
        for k in range(KT):
            nc.scalar.dma_start(wp_t[k][:], wp_d[k * 128:(k + 1) * 128, :])

        v_t = [pp.tile([128, VW], f16, tag=f"v{t}", name=f"v{t}")
               for t in range(TT)]
        qkT = [pp.tile([128, M], f16, tag=f"qk{n}", name=f"qk{n}")
               for n in range(NQK)]
        yT_t = [pp.tile([128, M], f16, tag=f"yT{c}", name=f"yT{c}")
                for c in range(KT)]
        for t in range(TT):
            nc.vector.memset(
                v_t[t].rearrange("p (h c) -> p h c", c=D + 1)
                    [:, :, D:D + 1], 1.0)

        def qkv_chain(b, i):
            """i in [0, 20): 12 QK n-tiles then 8 V half-tiles."""
            bcol = b * T
            if i < NQK:
                n = i
                p = ps_mm.tile([128, 512], f32, tag="mm", name=f"mm{b}_{i}")
                for k in range(KT):
                    nc.tensor.matmul(
                        p[:],
                        wa_t[k][:, n * 128:(n + 1) * 128],
                        xT[k][:, bcol:bcol + T],
                        start=(k == 0), stop=False)
                nc.tensor.matmul(
                    p[:],
                    ba_t[0:1, n * 128:(n + 1) * 128],
                    ones[0:1, bcol:bcol + T],
                    start=False, stop=True)
                nc.vector.tensor_copy(qkT[n][:, bcol:bcol + T], p[:])
            else:
                j = i - NQK
                t = b * 4 + j // 2
                lo, w = ((0, 512), (512, 256))[j % 2]
                p = ps_mm.tile([128, 512], f32, tag="mm", name=f"mm{b}_{i}")
                for k in range(KT):
                    nc.tensor.matmul(
                        p[:, :w],
                        xT[k][:, t * 128:(t + 1) * 128],
                        wa_t[k][:, 2 * C + lo:2 * C + lo + w],
                        start=(k == 0), stop=False)
                nc.tensor.matmul(
                    p[:, :w],
                    ones[0:1, t * 128:(t + 1) * 128],
                    ba_t[0:1, 2 * C + lo:2 * C + lo + w],
                    start=False, stop=True)
                h0 = lo // D
                nc.vector.tensor_copy(
                    v_t[t].rearrange("p (h c) -> p h c", c=D + 1)
                        [:, h0:h0 + w // D, 0:D],
                    p[:, :w].rearrange("p (h c) -> p h c", c=D))

        yun_all = {}
        r_tiles = {}

        def attention_hp(b, hp):
            bcol = b * T
            if hp == 0:
                r_tiles[b] = np_.tile([H, 512], f32, tag="r_all",
                                      name=f"r_all{b}")
            e_tiles = []
            for kt in range(4):
                ps = ps_s.tile([128, 1024], f32)
                for sub in range(2):
                    r0 = 64 * sub
                    nc.tensor.matmul(
                        ps[:, sub * 512:sub * 512 + 512],
                        qkT[6 + hp][r0:r0 + D,
                                    bcol + kt * 128:bcol + (kt + 1) * 128],
                        qkT[hp][r0:r0 + D, bcol:bcol + T],
                        start=True, stop=True)
                e = ap_.tile([128, 1024], f16, tag="e")
                nc.scalar.activation(
                    e[:], ps[:], Exp,
                    bias=mask_sb[:, b * 4 + kt:b * 4 + kt + 1],
                    scale=float(SCALE))
                e_tiles.append(e)
            for sub in range(2):
                h = 2 * hp + sub
                py = ps_y.tile([128, 512], f32)
                for kt in range(4):
                    nc.tensor.matmul(
                        py[0:D + 1, :],
                        v_t[b * 4 + kt][:, (D + 1) * h:(D + 1) * (h + 1)],
                        e_tiles[kt][:, sub * 512:sub * 512 + 512],
                        start=(kt == 0), stop=(kt == 3))
                yun = np_.tile([64, 512], f16, tag="yun", bufs=14,
                               name=f"yun{b}_{h}")
                nc.vector.tensor_copy(yun[:], py[0:D, :])
                rs = np_.tile([D + 1, 512], f32, tag="rstage")
                nc.scalar.copy(rs[D:D + 1, :], py[D:D + 1, :])
                nc.sync.dma_start(r_tiles[b][h:h + 1, :], rs[D:D + 1, :])
                yun_all[(b, h)] = yun

        def norm_tail(b):
            bcol = b * T
            recip = np_.tile([H, 512], f32, tag="recip")
            nc.vector.reciprocal(recip[:], r_tiles[b][:])
            recip16 = np_.tile([H, 512], f16, tag="recip16")
            nc.vector.tensor_copy(recip16[:], recip[:])
            for h in range(H):
                nt, r0 = h // 2, 64 * (h % 2)
                rep = np_.tile([64, 512], f16, tag="rep")
                nc.sync.dma_start(
                    rep[:],
                    recip16[h:h + 1, None, :].broadcast_to((1, 64, 512)))
                dst = yT_t[nt][r0:r0 + D, bcol:bcol + T]
                if r0 == 0:
                    nc.vector.tensor_mul(dst, yun_all[(b, h)][:], rep[:])
                else:
                    st = np_.tile([64, 512], f16, tag="stage")
                    nc.vector.tensor_mul(st[:], yun_all[(b, h)][:], rep[:])
                    nc.scalar.dma_start(dst, st[:])

        def proj_chunk(b, i):
            t = b * 4 + i // 2
            lo, w = ((0, 512), (512, 256))[i % 2]
            p = ps_mm.tile([128, 512], f32, tag="mm", name=f"pj{b}_{i}")
            for k in range(KT):
                nc.tensor.matmul(
                    p[:, :w],
                    yT_t[k][:, t * 128:(t + 1) * 128],
                    wp_t[k][:, lo:lo + w],
                    start=(k == 0), stop=False)
            nc.tensor.matmul(
                p[:, :w],
                ones[0:1, t * 128:(t + 1) * 128],
                bp_t[0:1, lo:lo + w],
                start=False, stop=True)
            ot = np_.tile([128, 512], f32, tag="ostage", bufs=3)
            nc.vector.tensor_copy(ot[:, :w], p[:, :w])
            nc.scalar.dma_start(
                out_d[t * 128:(t + 1) * 128, lo:lo + w], ot[:, :w])

        # software-pipelined emission
        for i in range(20):
            qkv_chain(0, i)
        qk1 = iter(range(20))
        for hp in range(6):
            attention_hp(0, hp)
            for _ in range(4 if hp < 2 else 3):
                i = next(qk1, None)
                if i is not None:
                    qkv_chain(1, i)
        norm_tail(0)
        pj0 = iter(range(8))
        for hp in range(6):
            attention_hp(1, hp)
            for _ in range(2 if hp < 2 else 1):
                i = next(pj0, None)
                if i is not None:
                    proj_chunk(0, i)
        norm_tail(1)
        for i in pj0:
            proj_chunk(0, i)
        for i in range(8):
            proj_chunk(1, i)

    nc.compile()
    return nc


def get_compiled():
    if "nc" not in _cache:
        _cache["nc"] = _build()
    return _cache["nc"]


def make_in_maps(x, attention_mask, W_attn, b_attn, W_proj, b_proj):
    x = np.asarray(x, dtype=np.float32).astype(np.float16)
    mask = np.ascontiguousarray(
        np.asarray(attention_mask, dtype=np.float32)[:, 0, 0, :])
    wa = np.asarray(W_attn, dtype=np.float32).astype(np.float16)
    ba = np.asarray(b_attn, dtype=np.float32).astype(np.float16).reshape(1, C3)
    wp = np.asarray(W_proj, dtype=np.float32).astype(np.float16)
    bp = np.asarray(b_proj, dtype=np.float32).astype(np.float16).reshape(1, C)
    maps = []
    for i in range(N_CORES):
        maps.append({
            "x": np.ascontiguousarray(x[BC * i:BC * (i + 1)].reshape(M, C)),
            "mask": np.ascontiguousarray(mask[BC * i:BC * (i + 1)]),
            "w_attn": wa, "b_attn": ba, "w_proj": wp, "b_proj": bp,
        })
    return maps


def kernel(x, attention_mask, W_attn, b_attn, W_proj, b_proj):
    from concourse.bass_utils import run_bass_kernel_spmd

    nc = get_compiled()
    in_maps = make_in_maps(x, attention_mask, W_attn, b_attn, W_proj, b_proj)
    last_err = None
    for _ in range(3):
        try:
            res = run_bass_kernel_spmd(nc, in_maps, list(range(N_CORES)))
            break
        except Exception as e:  # transient NRT device errors: retry
            last_err = e
    else:
        raise last_err
    out = np.concatenate(
        [res.results[i]["out"].reshape(BC, T, C) for i in range(N_CORES)], axis=0)
    return out.astype(np.float32)


# revision 18
# speedup vs baseline: 1.3617x; 1.0426x over previous
"""BERT self-attention (B=16, T=512, C=768, H=12, D=64) on 8 trn2 NeuronCores.

Data-parallel over batch: each core gets 2 batches. Matmul operands are fp16
(11-bit mantissa, ~tf32-class precision, 1 cycle/row PE streaming, FWL weight
loads); all accumulation stays fp32 in PSUM. Per core:
  xT    = x transposed during load via the DMA XBAR transpose (fp16).
  Q^T/K^T ([feature, token] layout, lhsT = W_attn tile) and V ([token, feature]
          layout with an interleaved ones column per head, lhsT = xT tile).
  S^T   = K^T-as-lhsT matmul -> scores in [key, query] layout (K=64, head pairs
          packed in PE row groups via base-partition-64 slices).
  P     = exp(S/8 + mask) on ScalarE (mask is a per-partition bias in this
          layout), written as fp16.
  y^T   = lhsT=[V_h | ones] matmul -> unnormalized y^T plus softmax row-sums as
          an extra PSUM row; row-sums are collected per batch, inverted in one
          batched DVE reciprocal, replicated across partitions by a
          broadcast-AP DMA, and applied with a DVE multiply.
  out   = y^T-as-lhsT matmul with W_proj + b_proj (fp32 result to DRAM).
Biases are folded in as K=1 accumulating matmuls against a ones row.
"""

import sys

sys.path.insert(0, "/opt/trn_rl_repo")

from contextlib import ExitStack

import numpy as np

B, T, C = 16, 512, 768
H, D = 12, 64
C3 = 3 * C
N_CORES = 8
BC = B // N_CORES           # batches per core
M = BC * T                  # tokens per core
KT = C // 128               # feature k-tiles (6)
TT = M // 128               # token tiles per core (8)
NQK = 2 * C // 128          # q+k feature n-tiles (12)
VW = H * (D + 1)            # v tile width with interleaved ones cols (780)
SCALE = 1.0 / np.sqrt(D)

_cache = {}


def _build():
    import concourse.bass as bass
    import concourse.tile as tile
    from concourse import bacc, mybir
    f32 = mybir.dt.float32
    f16 = mybir.dt.float16
    Exp = mybir.ActivationFunctionType.Exp

    nc = bacc.Bacc("TRN2", target_bir_lowering=False, debug=False,
                   num_devices=N_CORES)
    x_d = nc.dram_tensor("x", [M, C], f16, kind="ExternalInput").ap()
    mask_d = nc.dram_tensor("mask", [BC, T], f32, kind="ExternalInput").ap()
    wa_d = nc.dram_tensor("w_attn", [C, C3], f16, kind="ExternalInput").ap()
    ba_d = nc.dram_tensor("b_attn", [1, C3], f16, kind="ExternalInput").ap()
    wp_d = nc.dram_tensor("w_proj", [C, C], f16, kind="ExternalInput").ap()
    bp_d = nc.dram_tensor("b_proj", [1, C], f16, kind="ExternalInput").ap()
    out_d = nc.dram_tensor("out", [M, C], f32, kind="ExternalOutput").ap()

    with tile.TileContext(nc) as tc, ExitStack() as ctx:
        pp = ctx.enter_context(tc.tile_pool(name="pp", bufs=1))
        np_ = ctx.enter_context(tc.tile_pool(name="norm", bufs=4))
        ap_ = ctx.enter_context(tc.tile_pool(name="att", bufs=6))
        ps_mm = ctx.enter_context(tc.tile_pool(name="ps_mm", bufs=2, space="PSUM"))
        ps_s = ctx.enter_context(tc.tile_pool(name="ps_s", bufs=2, space="PSUM"))
        ps_y = ctx.enter_context(tc.tile_pool(name="ps_y", bufs=2, space="PSUM"))

        ones = pp.tile([1, M], f16, tag="ones")
        nc.vector.memset(ones[:], 1.0)
        mask_sb = pp.tile([128, BC * 4], f32, tag="mask")
        nc.gpsimd.dma_start(
            mask_sb[:],
            mask_d.rearrange("a b -> (a b)").rearrange("(j p) -> p j", p=128))
        ba_t = pp.tile([1, C3], f16, tag="ba")
        nc.gpsimd.dma_start(ba_t[:], ba_d[:])
        bp_t = pp.tile([1, C], f16, tag="bp")
        nc.gpsimd.dma_start(bp_t[:], bp_d[:])

        wa_t = [pp.tile([128, C3], f16, tag=f"wa{k}", name=f"wa{k}")
                for k in range(KT)]
        xT = [pp.tile([128, M], f16, tag=f"xT{k}", name=f"xT{k}")
              for k in range(KT)]
        for k in range(KT):
            qx = nc.sync if k % 2 == 0 else nc.scalar
            qw = nc.scalar if k % 2 == 0 else nc.sync
            qx.dma_start(
                xT[k][:], x_d[:, k * 128:(k + 1) * 128], transpose=True)
            qw.dma_start(wa_t[k][:], wa_d[k * 128:(k + 1) * 128, :])
        wp_t = [pp.tile([128, C], f16, tag=f"wp{k}", name=f"wp{k}")
                for k in range(KT)]
        for k in range(KT):
            nc.gpsimd.dma_start(wp_t[k][:], wp_d[k * 128:(k + 1) * 128, :])

        v_t = [pp.tile([128, VW], f16, tag=f"v{t}", name=f"v{t}")
               for t in range(TT)]
        qkT = [pp.tile([128, M], f16, tag=f"qk{n}", name=f"qk{n}")
               for n in range(NQK)]
        yT_t = [pp.tile([128, M], f16, tag=f"yT{c}", name=f"yT{c}")
                for c in range(KT)]
        for t in range(TT):
            nc.vector.memset(
                v_t[t].rearrange("p (h c) -> p h c", c=D + 1)
                    [:, :, D:D + 1], 1.0)

        def qkv_chain(b, i):
            """i in [0, 20): 12 QK n-tiles then 8 V half-tiles."""
            bcol = b * T
            if i < NQK:
                n = i
                p = ps_mm.tile([128, 512], f32, tag="mm", name=f"mm{b}_{i}")
                for k in range(KT):
                    nc.tensor.matmul(
                        p[:],
                        wa_t[k][:, n * 128:(n + 1) * 128],
                        xT[k][:, bcol:bcol + T],
                        start=(k == 0), stop=False)
                nc.tensor.matmul(
                    p[:],
                    ba_t[0:1, n * 128:(n + 1) * 128],
                    ones[0:1, bcol:bcol + T],
                    start=False, stop=True)
                nc.vector.tensor_copy(qkT[n][:, bcol:bcol + T], p[:])
            else:
                j = i - NQK
                t = b * 4 + j // 2
                lo, w = ((0, 512), (512, 256))[j % 2]
                p = ps_mm.tile([128, 512], f32, tag="mm", name=f"mm{b}_{i}")
                for k in range(KT):
                    nc.tensor.matmul(
                        p[:, :w],
                        xT[k][:, t * 128:(t + 1) * 128],
                        wa_t[k][:, 2 * C + lo:2 * C + lo + w],
                        start=(k == 0), stop=False)
                nc.tensor.matmul(
                    p[:, :w],
                    ones[0:1, t * 128:(t + 1) * 128],
                    ba_t[0:1, 2 * C + lo:2 * C + lo + w],
                    start=False, stop=True)
                h0 = lo // D
                nc.vector.tensor_copy(
                    v_t[t].rearrange("p (h c) -> p h c", c=D + 1)
                        [:, h0:h0 + w // D, 0:D],
                    p[:, :w].rearrange("p (h c) -> p h c", c=D))

        yun_all = {}
        r_tiles = {}

        def attention_hp(b, hp):
            bcol = b * T
            if hp % 3 == 0:
                r_tiles[(b, hp // 3)] = np_.tile(
                    [H // 2, 512], f32, tag="r_all", bufs=4,
                    name=f"r_all{b}_{hp // 3}")
            e_tiles = []
            for kt in range(4):
                ps = ps_s.tile([128, 1024], f32)
                for sub in range(2):
                    r0 = 64 * sub
                    nc.tensor.matmul(
                        ps[:, sub * 512:sub * 512 + 512],
                        qkT[6 + hp][r0:r0 + D,
                                    bcol + kt * 128:bcol + (kt + 1) * 128],
                        qkT[hp][r0:r0 + D, bcol:bcol + T],
                        start=True, stop=True)
                e = ap_.tile([128, 1024], f16, tag="e")
                nc.scalar.activation(
                    e[:], ps[:], Exp,
                    bias=mask_sb[:, b * 4 + kt:b * 4 + kt + 1],
                    scale=float(SCALE))
                e_tiles.append(e)
            for sub in range(2):
                h = 2 * hp + sub
                py = ps_y.tile([128, 512], f32)
                for kt in range(4):
                    nc.tensor.matmul(
                        py[0:D + 1, :],
                        v_t[b * 4 + kt][:, (D + 1) * h:(D + 1) * (h + 1)],
                        e_tiles[kt][:, sub * 512:sub * 512 + 512],
                        start=(kt == 0), stop=(kt == 3))
                yun = np_.tile([64, 512], f16, tag="yun", bufs=14,
                               name=f"yun{b}_{h}")
                nc.vector.tensor_copy(yun[:], py[0:D, :])
                rs = np_.tile([D + 1, 512], f32, tag="rstage")
                nc.scalar.copy(rs[D:D + 1, :], py[D:D + 1, :])
                nc.sync.dma_start(
                    r_tiles[(b, hp // 3)][h % 6:h % 6 + 1, :], rs[D:D + 1, :])
                yun_all[(b, h)] = yun

        def norm_tail(b, half):
            bcol = b * T
            recip = np_.tile([H // 2, 512], f32, tag="recip")
            nc.vector.reciprocal(recip[:], r_tiles[(b, half)][:])
            recip16 = np_.tile([H // 2, 512], f16, tag="recip16")
            nc.vector.tensor_copy(recip16[:], recip[:])
            for h in range(6 * half, 6 * half + 6):
                nt, r0 = h // 2, 64 * (h % 2)
                rep = np_.tile([64, 512], f16, tag="rep")
                q = nc.sync if h % 2 == 0 else nc.scalar
                q.dma_start(
                    rep[:],
                    recip16[h % 6:h % 6 + 1, None, :]
                    .broadcast_to((1, 64, 512)))
                dst = yT_t[nt][r0:r0 + D, bcol:bcol + T]
                if r0 == 0:
                    nc.vector.tensor_mul(dst, yun_all[(b, h)][:], rep[:])
                else:
                    st = np_.tile([64, 512], f16, tag="stage")
                    nc.vector.tensor_mul(st[:], yun_all[(b, h)][:], rep[:])
                    nc.gpsimd.dma_start(dst, st[:])

        def proj_chunk(b, i):
            t = b * 4 + i // 2
            lo, w = ((0, 512), (512, 256))[i % 2]
            p = ps_mm.tile([128, 512], f32, tag="mm", name=f"pj{b}_{i}")
            for k in range(KT):
                nc.tensor.matmul(
                    p[:, :w],
                    yT_t[k][:, t * 128:(t + 1) * 128],
                    wp_t[k][:, lo:lo + w],
                    start=(k == 0), stop=False)
            nc.tensor.matmul(
                p[:, :w],
                ones[0:1, t * 128:(t + 1) * 128],
                bp_t[0:1, lo:lo + w],
                start=False, stop=True)
            ot = np_.tile([128, 512], f32, tag="ostage", bufs=3)
            nc.vector.tensor_copy(ot[:, :w], p[:, :w])
            nc.gpsimd.dma_start(
                out_d[t * 128:(t + 1) * 128, lo:lo + w], ot[:, :w])

        # software-pipelined emission
        for i in range(20):
            qkv_chain(0, i)
        qk1 = iter(range(20))
        for hp in range(6):
            attention_hp(0, hp)
            if hp == 3:
                norm_tail(0, 0)
            for _ in range(4 if hp < 2 else 3):
                i = next(qk1, None)
                if i is not None:
                    qkv_chain(1, i)
        norm_tail(0, 1)
        pj0 = iter(range(8))
        for hp in range(6):
            attention_hp(1, hp)
            if hp == 3:
                norm_tail(1, 0)
            for _ in range(2 if hp < 2 else 1):
                i = next(pj0, None)
                if i is not None:
                    proj_chunk(0, i)
        norm_tail(1, 1)
        for i in pj0:
            proj_chunk(0, i)
        for i in range(8):
            proj_chunk(1, i)

    nc.compile()
    return nc


def get_compiled():
    if "nc" not in _cache:
        _cache["nc"] = _build()
    return _cache["nc"]


def make_in_maps(x, attention_mask, W_attn, b_attn, W_proj, b_proj):
    x = np.asarray(x, dtype=np.float32).astype(np.float16)
    mask = np.ascontiguousarray(
        np.asarray(attention_mask, dtype=np.float32)[:, 0, 0, :])
    wa = np.asarray(W_attn, dtype=np.float32).astype(np.float16)
    ba = np.asarray(b_attn, dtype=np.float32).astype(np.float16).reshape(1, C3)
    wp = np.asarray(W_proj, dtype=np.float32).astype(np.float16)
    bp = np.asarray(b_proj, dtype=np.float32).astype(np.float16).reshape(1, C)
    maps = []
    for i in range(N_CORES):
        maps.append({
            "x": np.ascontiguousarray(x[BC * i:BC * (i + 1)].reshape(M, C)),
            "mask": np.ascontiguousarray(mask[BC * i:BC * (i + 1)]),
            "w_attn": wa, "b_attn": ba, "w_proj": wp, "b_proj": bp,
        })
    return maps


def kernel(x, attention_mask, W_attn, b_attn, W_proj, b_proj):
    from concourse.bass_utils import run_bass_kernel_spmd

    nc = get_compiled()
    in_maps = make_in_maps(x, attention_mask, W_attn, b_attn, W_proj, b_proj)
    last_err = None
    for _ in range(3):
        try:
            res = run_bass_kernel_spmd(nc, in_maps, list(range(N_CORES)))
            break
        except Exception as e:  # transient NRT device errors: retry
            last_err = e
    else:
        raise last_err
    out = np.concatenate(
        [res.results[i]["out"].reshape(BC, T, C) for i in range(N_CORES)], axis=0)
    return out.astype(np.float32)


# revision 22
# speedup vs baseline: 1.3744x; 1.0093x over previous
"""BERT self-attention (B=16, T=512, C=768, H=12, D=64) on 8 trn2 NeuronCores.

Data-parallel over batch: each core gets 2 batches. Matmul operands are fp16
(11-bit mantissa, ~tf32-class precision, 1 cycle/row PE streaming, FWL weight
loads); all accumulation stays fp32 in PSUM. Per core:
  xT    = x transposed during load via the DMA XBAR transpose (fp16).
  Q^T/K^T ([feature, token] layout, lhsT = W_attn tile) and V ([token, feature]
          layout with an interleaved ones column per head, lhsT = xT tile).
  S^T   = K^T-as-lhsT matmul -> scores in [key, query] layout (K=64, head pairs
          packed in PE row groups via base-partition-64 slices).
  P     = exp(S/8 + mask) on ScalarE (mask is a per-partition bias in this
          layout), written as fp16.
  y^T   = lhsT=[V_h | ones] matmul -> unnormalized y^T plus softmax row-sums as
          an extra PSUM row; row-sums are collected per batch, inverted in one
          batched DVE reciprocal, replicated across partitions by a
          broadcast-AP DMA, and applied with a DVE multiply.
  out   = y^T-as-lhsT matmul with W_proj + b_proj (fp32 result to DRAM).
Biases are folded in as K=1 accumulating matmuls against a ones row.
"""

import sys

sys.path.insert(0, "/opt/trn_rl_repo")

from contextlib import ExitStack

import numpy as np

B, T, C = 16, 512, 768
H, D = 12, 64
C3 = 3 * C
N_CORES = 8
BC = B // N_CORES           # batches per core
M = BC * T                  # tokens per core
KT = C // 128               # feature k-tiles (6)
TT = M // 128               # token tiles per core (8)
NQK = 2 * C // 128          # q+k feature n-tiles (12)
VW = H * (D + 1)            # v tile width with interleaved ones cols (780)
SCALE = 1.0 / np.sqrt(D)

_cache = {}


def _build():
    import concourse.bass as bass
    import concourse.tile as tile
    from concourse import bacc, mybir
    f32 = mybir.dt.float32
    f16 = mybir.dt.float16
    Exp = mybir.ActivationFunctionType.Exp

    nc = bacc.Bacc("TRN2", target_bir_lowering=False, debug=False,
                   num_devices=N_CORES)
    x_d = nc.dram_tensor("x", [M, C], f16, kind="ExternalInput").ap()
    mask_d = nc.dram_tensor("mask", [BC, T], f32, kind="ExternalInput").ap()
    wa_d = nc.dram_tensor("w_attn", [C, C3], f16, kind="ExternalInput").ap()
    ba_d = nc.dram_tensor("b_attn", [1, C3], f16, kind="ExternalInput").ap()
    wp_d = nc.dram_tensor("w_proj", [C, C], f16, kind="ExternalInput").ap()
    bp_d = nc.dram_tensor("b_proj", [1, C], f16, kind="ExternalInput").ap()
    out_d = nc.dram_tensor("out", [M, C], f32, kind="ExternalOutput").ap()

    with tile.TileContext(nc) as tc, ExitStack() as ctx:
        pp = ctx.enter_context(tc.tile_pool(name="pp", bufs=1))
        np_ = ctx.enter_context(tc.tile_pool(name="norm", bufs=4))
        ap_ = ctx.enter_context(tc.tile_pool(name="att", bufs=6))
        ps_mm = ctx.enter_context(tc.tile_pool(name="ps_mm", bufs=2, space="PSUM"))
        ps_s = ctx.enter_context(tc.tile_pool(name="ps_s", bufs=2, space="PSUM"))
        ps_y = ctx.enter_context(tc.tile_pool(name="ps_y", bufs=2, space="PSUM"))

        ones = pp.tile([1, M], f16, tag="ones")
        nc.vector.memset(ones[:], 1.0)
        mask_sb = pp.tile([128, BC * 4], f32, tag="mask")
        nc.gpsimd.dma_start(
            mask_sb[:],
            mask_d.rearrange("a b -> (a b)").rearrange("(j p) -> p j", p=128))
        ba_t = pp.tile([1, C3], f16, tag="ba")
        nc.gpsimd.dma_start(ba_t[:], ba_d[:])
        bp_t = pp.tile([1, C], f16, tag="bp")
        nc.gpsimd.dma_start(bp_t[:], bp_d[:])

        wa_t = [pp.tile([128, C3], f16, tag=f"wa{k}", name=f"wa{k}")
                for k in range(KT)]
        xT = [pp.tile([128, M], f16, tag=f"xT{k}", name=f"xT{k}")
              for k in range(KT)]
        for k in range(KT):
            qx = nc.sync if k % 2 == 0 else nc.scalar
            qw = nc.scalar if k % 2 == 0 else nc.sync
            qx.dma_start(
                xT[k][:], x_d[:, k * 128:(k + 1) * 128], transpose=True)
            qw.dma_start(wa_t[k][:], wa_d[k * 128:(k + 1) * 128, :])
        wp_t = [pp.tile([128, C], f16, tag=f"wp{k}", name=f"wp{k}")
                for k in range(KT)]
        for k in range(KT):
            nc.gpsimd.dma_start(wp_t[k][:], wp_d[k * 128:(k + 1) * 128, :])

        v_t = [pp.tile([128, VW], f16, tag=f"v{t}", name=f"v{t}")
               for t in range(TT)]
        qkT = [pp.tile([128, M], f16, tag=f"qk{n}", name=f"qk{n}")
               for n in range(NQK)]
        yT_t = [pp.tile([128, M], f16, tag=f"yT{c}", name=f"yT{c}")
                for c in range(KT)]
        for t in range(TT):
            nc.vector.memset(
                v_t[t].rearrange("p (h c) -> p h c", c=D + 1)
                    [:, :, D:D + 1], 1.0)

        def qkv_chain(b, i):
            """i in [0, 20): 12 QK n-tiles then 8 V half-tiles."""
            bcol = b * T
            if i < NQK:
                n = i
                p = ps_mm.tile([128, 512], f32, tag="mm", name=f"mm{b}_{i}")
                for k in range(KT):
                    nc.tensor.matmul(
                        p[:],
                        wa_t[k][:, n * 128:(n + 1) * 128],
                        xT[k][:, bcol:bcol + T],
                        start=(k == 0), stop=False)
                nc.tensor.matmul(
                    p[:],
                    ba_t[0:1, n * 128:(n + 1) * 128],
                    ones[0:1, bcol:bcol + T],
                    start=False, stop=True)
                nc.vector.tensor_copy(qkT[n][:, bcol:bcol + T], p[:])
            else:
                j = i - NQK
                t = b * 4 + j // 2
                lo, w = ((0, 512), (512, 256))[j % 2]
                p = ps_mm.tile([128, 512], f32, tag="mm", name=f"mm{b}_{i}")
                for k in range(KT):
                    nc.tensor.matmul(
                        p[:, :w],
                        xT[k][:, t * 128:(t + 1) * 128],
                        wa_t[k][:, 2 * C + lo:2 * C + lo + w],
                        start=(k == 0), stop=False)
                nc.tensor.matmul(
                    p[:, :w],
                    ones[0:1, t * 128:(t + 1) * 128],
                    ba_t[0:1, 2 * C + lo:2 * C + lo + w],
                    start=False, stop=True)
                h0 = lo // D
                nc.vector.tensor_copy(
                    v_t[t].rearrange("p (h c) -> p h c", c=D + 1)
                        [:, h0:h0 + w // D, 0:D],
                    p[:, :w].rearrange("p (h c) -> p h c", c=D))

        yun_all = {}
        r_tiles = {}

        def attention_hp(b, hp):
            bcol = b * T
            if hp % 3 == 0:
                r_tiles[(b, hp // 3)] = np_.tile(
                    [H // 2, 512], f32, tag="r_all", bufs=4,
                    name=f"r_all{b}_{hp // 3}")
            e_tiles = []
            for kt in range(4):
                ps = ps_s.tile([128, 1024], f32)
                for sub in range(2):
                    r0 = 64 * sub
                    nc.tensor.matmul(
                        ps[:, sub * 512:sub * 512 + 512],
                        qkT[6 + hp][r0:r0 + D,
                                    bcol + kt * 128:bcol + (kt + 1) * 128],
                        qkT[hp][r0:r0 + D, bcol:bcol + T],
                        start=True, stop=True)
                e = ap_.tile([128, 1024], f16, tag="e")
                nc.scalar.activation(
                    e[:], ps[:], Exp,
                    bias=mask_sb[:, b * 4 + kt:b * 4 + kt + 1],
                    scale=float(SCALE))
                e_tiles.append(e)
            for sub in range(2):
                h = 2 * hp + sub
                py = ps_y.tile([128, 512], f32)
                for kt in range(4):
                    nc.tensor.matmul(
                        py[0:D + 1, :],
                        v_t[b * 4 + kt][:, (D + 1) * h:(D + 1) * (h + 1)],
                        e_tiles[kt][:, sub * 512:sub * 512 + 512],
                        start=(kt == 0), stop=(kt == 3))
                yun = np_.tile([64, 512], f16, tag="yun", bufs=14,
                               name=f"yun{b}_{h}")
                nc.vector.tensor_copy(yun[:], py[0:D, :])
                rs = np_.tile([D + 1, 512], f32, tag="rstage")
                nc.scalar.copy(rs[D:D + 1, :], py[D:D + 1, :])
                nc.sync.dma_start(
                    r_tiles[(b, hp // 3)][h % 6:h % 6 + 1, :], rs[D:D + 1, :])
                yun_all[(b, h)] = yun

        def norm_tail(b, half):
            bcol = b * T
            recip = np_.tile([H // 2, 512], f32, tag="recip")
            nc.vector.reciprocal(recip[:], r_tiles[(b, half)][:])
            recip16 = np_.tile([H // 2, 512], f16, tag="recip16")
            nc.vector.tensor_copy(recip16[:], recip[:])
            for h in range(6 * half, 6 * half + 6):
                nt, r0 = h // 2, 64 * (h % 2)
                rep = np_.tile([64, 512], f16, tag="rep")
                q = nc.sync if h % 2 == 0 else nc.scalar
                q.dma_start(
                    rep[:],
                    recip16[h % 6:h % 6 + 1, None, :]
                    .broadcast_to((1, 64, 512)))
                dst = yT_t[nt][r0:r0 + D, bcol:bcol + T]
                if r0 == 0:
                    nc.vector.tensor_mul(dst, yun_all[(b, h)][:], rep[:])
                else:
                    st = np_.tile([64, 512], f16, tag="stage")
                    nc.vector.tensor_mul(st[:], yun_all[(b, h)][:], rep[:])
                    nc.gpsimd.dma_start(dst, st[:])

        pj_part = {}

        def proj_chunk(b, i, ks=0, ke=KT, partial=False):
            t = b * 4 + i // 2
            lo, w = ((0, 512), (512, 256))[i % 2]
            p = ps_mm.tile([128, 512], f32, tag="mm", name=f"pj{b}_{i}_{ks}")
            for k in range(ks, ke):
                nc.tensor.matmul(
                    p[:, :w],
                    yT_t[k][:, t * 128:(t + 1) * 128],
                    wp_t[k][:, lo:lo + w],
                    start=(k == ks), stop=(partial and k == ke - 1))
            if partial:
                pt = np_.tile([128, 512], f32, tag="pjpart", bufs=8,
                              name=f"pjpart{i}")
                nc.vector.tensor_copy(pt[:, :w], p[:, :w])
                pj_part[(b, i)] = pt
                return
            nc.tensor.matmul(
                p[:, :w],
                ones[0:1, t * 128:(t + 1) * 128],
                bp_t[0:1, lo:lo + w],
                start=False, stop=True)
            ot = np_.tile([128, 512], f32, tag="ostage", bufs=3)
            if (b, i) in pj_part:
                nc.vector.tensor_tensor(
                    out=ot[:, :w], in0=p[:, :w], in1=pj_part[(b, i)][:, :w],
                    op=mybir.AluOpType.add)
            else:
                nc.vector.tensor_copy(ot[:, :w], p[:, :w])
            nc.gpsimd.dma_start(
                out_d[t * 128:(t + 1) * 128, lo:lo + w], ot[:, :w])

        # software-pipelined emission
        for i in range(20):
            qkv_chain(0, i)
        qk1 = iter(range(20))
        for hp in range(6):
            attention_hp(0, hp)
            if hp == 3:
                norm_tail(0, 0)
            for _ in range(4 if hp < 2 else 3):
                i = next(qk1, None)
                if i is not None:
                    qkv_chain(1, i)
        norm_tail(0, 1)
        pj0 = iter(range(8))
        for hp in range(6):
            attention_hp(1, hp)
            if hp == 3:
                norm_tail(1, 0)
                for i in range(8):
                    proj_chunk(1, i, 0, KT // 2, partial=True)
            i = next(pj0, None)
            if i is not None:
                proj_chunk(0, i)
        norm_tail(1, 1)
        for i in pj0:
            proj_chunk(0, i)
        for i in range(8):
            proj_chunk(1, i, KT // 2, KT)

    nc.compile()
    return nc


def get_compiled():
    if "nc" not in _cache:
        _cache["nc"] = _build()
    return _cache["nc"]


def make_in_maps(x, attention_mask, W_attn, b_attn, W_proj, b_proj):
    x = np.asarray(x, dtype=np.float32).astype(np.float16)
    mask = np.ascontiguousarray(
        np.asarray(attention_mask, dtype=np.float32)[:, 0, 0, :])
    wa = np.asarray(W_attn, dtype=np.float32).astype(np.float16)
    ba = np.asarray(b_attn, dtype=np.float32).astype(np.float16).reshape(1, C3)
    wp = np.asarray(W_proj, dtype=np.float32).astype(np.float16)
    bp = np.asarray(b_proj, dtype=np.float32).astype(np.float16).reshape(1, C)
    maps = []
    for i in range(N_CORES):
        maps.append({
            "x": np.ascontiguousarray(x[BC * i:BC * (i + 1)].reshape(M, C)),
            "mask": np.ascontiguousarray(mask[BC * i:BC * (i + 1)]),
            "w_attn": wa, "b_attn": ba, "w_proj": wp, "b_proj": bp,
        })
    return maps


def kernel(x, attention_mask, W_attn, b_attn, W_proj, b_proj):
    from concourse.bass_utils import run_bass_kernel_spmd

    nc = get_compiled()
    in_maps = make_in_maps(x, attention_mask, W_attn, b_attn, W_proj, b_proj)
    last_err = None
    for _ in range(3):
        try:
            res = run_bass_kernel_spmd(nc, in_maps, list(range(N_CORES)))
            break
        except Exception as e:  # transient NRT device errors: retry
            last_err = e
    else:
        raise last_err
    out = np.concatenate(
        [res.results[i]["out"].reshape(BC, T, C) for i in range(N_CORES)], axis=0)
    return out.astype(np.float32)


# revision 24
# speedup vs baseline: 1.4272x; 1.0384x over previous
"""BERT self-attention (B=16, T=512, C=768, H=12, D=64) on 8 trn2 NeuronCores.

Data-parallel over batch: each core gets 2 batches. Matmul operands are fp16
(11-bit mantissa, ~tf32-class precision, 1 cycle/row PE streaming, FWL weight
loads); all accumulation stays fp32 in PSUM. Per core:
  xT    = x transposed during load via the DMA XBAR transpose (fp16).
  Q^T/K^T ([feature, token] layout, lhsT = W_attn tile) and V ([token, feature]
          layout with an interleaved ones column per head, lhsT = xT tile).
  S^T   = K^T-as-lhsT matmul -> scores in [key, query] layout (K=64, head pairs
          packed in PE row groups via base-partition-64 slices).
  P     = exp(S/8 + mask) on ScalarE (mask is a per-partition bias in this
          layout), written as fp16.
  y^T   = lhsT=[V_h | ones] matmul -> unnormalized y^T plus softmax row-sums as
          an extra PSUM row; row-sums are collected per batch, inverted in one
          batched DVE reciprocal, replicated across partitions by a
          broadcast-AP DMA, and applied with a DVE multiply.
  out   = y^T-as-lhsT matmul with W_proj + b_proj (fp32 result to DRAM).
Biases are folded in as K=1 accumulating matmuls against a ones row.
"""

import sys

sys.path.insert(0, "/opt/trn_rl_repo")

from contextlib import ExitStack

import numpy as np

B, T, C = 16, 512, 768
H, D = 12, 64
C3 = 3 * C
N_CORES = 8
BC = B // N_CORES           # batches per core
M = BC * T                  # tokens per core
KT = C // 128               # feature k-tiles (6)
TT = M // 128               # token tiles per core (8)
NQK = 2 * C // 128          # q+k feature n-tiles (12)
VW = H * (D + 1)            # v tile width with interleaved ones cols (780)
SCALE = 1.0 / np.sqrt(D)

_cache = {}


def _build():
    import concourse.bass as bass
    import concourse.tile as tile
    from concourse import bacc, mybir
    f32 = mybir.dt.float32
    f16 = mybir.dt.float16
    Exp = mybir.ActivationFunctionType.Exp

    nc = bacc.Bacc("TRN2", target_bir_lowering=False, debug=False,
                   num_devices=N_CORES)
    x_d = nc.dram_tensor("x", [M, C], f16, kind="ExternalInput").ap()
    mask_d = nc.dram_tensor("mask", [BC, T], f32, kind="ExternalInput").ap()
    wa_d = nc.dram_tensor("w_attn", [C, C3], f16, kind="ExternalInput").ap()
    ba_d = nc.dram_tensor("b_attn", [1, C3], f16, kind="ExternalInput").ap()
    wp_d = nc.dram_tensor("w_proj", [C, C], f16, kind="ExternalInput").ap()
    bp_d = nc.dram_tensor("b_proj", [1, C], f16, kind="ExternalInput").ap()
    out_d = nc.dram_tensor("out", [M, C], f32, kind="ExternalOutput").ap()

    with tile.TileContext(nc) as tc, ExitStack() as ctx:
        pp = ctx.enter_context(tc.tile_pool(name="pp", bufs=1))
        np_ = ctx.enter_context(tc.tile_pool(name="norm", bufs=4))
        ap_ = ctx.enter_context(tc.tile_pool(name="att", bufs=6))
        ps_mm = ctx.enter_context(tc.tile_pool(name="ps_mm", bufs=2, space="PSUM"))
        ps_s = ctx.enter_context(tc.tile_pool(name="ps_s", bufs=2, space="PSUM"))
        ps_y = ctx.enter_context(tc.tile_pool(name="ps_y", bufs=2, space="PSUM"))

        ones = pp.tile([1, M], f16, tag="ones")
        nc.vector.memset(ones[:], 1.0)
        mask_sb = pp.tile([128, BC * 4], f32, tag="mask")
        nc.gpsimd.dma_start(
            mask_sb[:],
            mask_d.rearrange("a b -> (a b)").rearrange("(j p) -> p j", p=128))
        ba_t = pp.tile([1, C3], f16, tag="ba")
        nc.gpsimd.dma_start(ba_t[:], ba_d[:])
        bp_t = pp.tile([1, C], f16, tag="bp")
        nc.gpsimd.dma_start(bp_t[:], bp_d[:])

        wa_t = [pp.tile([128, C3], f16, tag=f"wa{k}", name=f"wa{k}")
                for k in range(KT)]
        xT = [pp.tile([128, M], f16, tag=f"xT{k}", name=f"xT{k}")
              for k in range(KT)]
        for k in range(KT):
            qx = nc.sync if k % 2 == 0 else nc.scalar
            qx.dma_start(
                xT[k][:], x_d[:, k * 128:(k + 1) * 128], transpose=True)
        for j in range(2):
            for k in range(KT):
                qw = nc.scalar if k % 2 == 0 else nc.sync
                qw.dma_start(
                    wa_t[k][:, j * 1152:(j + 1) * 1152],
                    wa_d[k * 128:(k + 1) * 128, j * 1152:(j + 1) * 1152])
        wp_t = [pp.tile([128, C], f16, tag=f"wp{k}", name=f"wp{k}")
                for k in range(KT)]
        for k in range(KT):
            nc.gpsimd.dma_start(wp_t[k][:], wp_d[k * 128:(k + 1) * 128, :])

        v_t = [pp.tile([128, VW], f16, tag=f"v{t}", name=f"v{t}")
               for t in range(TT)]
        qkT = [pp.tile([128, M], f16, tag=f"qk{n}", name=f"qk{n}")
               for n in range(NQK)]
        yT_t = [pp.tile([128, M], f16, tag=f"yT{c}", name=f"yT{c}")
                for c in range(KT)]
        for t in range(TT):
            nc.vector.memset(
                v_t[t].rearrange("p (h c) -> p h c", c=D + 1)
                    [:, :, D:D + 1], 1.0)

        def qkv_chain(b, i):
            """i in [0, 20): 12 QK n-tiles then 8 V half-tiles."""
            bcol = b * T
            if i < NQK:
                n = i
                p = ps_mm.tile([128, 512], f32, tag="mm", name=f"mm{b}_{i}")
                for k in range(KT):
                    nc.tensor.matmul(
                        p[:],
                        wa_t[k][:, n * 128:(n + 1) * 128],
                        xT[k][:, bcol:bcol + T],
                        start=(k == 0), stop=False)
                nc.tensor.matmul(
                    p[:],
                    ba_t[0:1, n * 128:(n + 1) * 128],
                    ones[0:1, bcol:bcol + T],
                    start=False, stop=True)
                nc.vector.tensor_copy(qkT[n][:, bcol:bcol + T], p[:])
            else:
                j = i - NQK
                t = b * 4 + j // 2
                lo, w = ((0, 512), (512, 256))[j % 2]
                p = ps_mm.tile([128, 512], f32, tag="mm", name=f"mm{b}_{i}")
                for k in range(KT):
                    nc.tensor.matmul(
                        p[:, :w],
                        xT[k][:, t * 128:(t + 1) * 128],
                        wa_t[k][:, 2 * C + lo:2 * C + lo + w],
                        start=(k == 0), stop=False)
                nc.tensor.matmul(
                    p[:, :w],
                    ones[0:1, t * 128:(t + 1) * 128],
                    ba_t[0:1, 2 * C + lo:2 * C + lo + w],
                    start=False, stop=True)
                h0 = lo // D
                nc.vector.tensor_copy(
                    v_t[t].rearrange("p (h c) -> p h c", c=D + 1)
                        [:, h0:h0 + w // D, 0:D],
                    p[:, :w].rearrange("p (h c) -> p h c", c=D))

        yun_all = {}
        r_tiles = {}

        def attention_hp(b, hp):
            bcol = b * T
            if hp % 3 == 0:
                r_tiles[(b, hp // 3)] = np_.tile(
                    [H // 2, 512], f32, tag="r_all", bufs=4,
                    name=f"r_all{b}_{hp // 3}")
            e_tiles = []
            for kt in range(4):
                ps = ps_s.tile([128, 1024], f32)
                for sub in range(2):
                    r0 = 64 * sub
                    nc.tensor.matmul(
                        ps[:, sub * 512:sub * 512 + 512],
                        qkT[6 + hp][r0:r0 + D,
                                    bcol + kt * 128:bcol + (kt + 1) * 128],
                        qkT[hp][r0:r0 + D, bcol:bcol + T],
                        start=True, stop=True)
                e = ap_.tile([128, 1024], f16, tag="e")
                nc.scalar.activation(
                    e[:], ps[:], Exp,
                    bias=mask_sb[:, b * 4 + kt:b * 4 + kt + 1],
                    scale=float(SCALE))
                e_tiles.append(e)
            for sub in range(2):
                h = 2 * hp + sub
                py = ps_y.tile([128, 512], f32)
                for kt in range(4):
                    nc.tensor.matmul(
                        py[0:D + 1, :],
                        v_t[b * 4 + kt][:, (D + 1) * h:(D + 1) * (h + 1)],
                        e_tiles[kt][:, sub * 512:sub * 512 + 512],
                        start=(kt == 0), stop=(kt == 3))
                yun = np_.tile([64, 512], f16, tag="yun", bufs=14,
                               name=f"yun{b}_{h}")
                nc.vector.tensor_copy(yun[:], py[0:D, :])
                rs = np_.tile([D + 1, 512], f32, tag="rstage")
                nc.scalar.copy(rs[D:D + 1, :], py[D:D + 1, :])
                nc.sync.dma_start(
                    r_tiles[(b, hp // 3)][h % 6:h % 6 + 1, :], rs[D:D + 1, :])
                yun_all[(b, h)] = yun

        def norm_tail(b, half):
            bcol = b * T
            recip = np_.tile([H // 2, 512], f32, tag="recip")
            nc.vector.reciprocal(recip[:], r_tiles[(b, half)][:])
            recip16 = np_.tile([H // 2, 512], f16, tag="recip16")
            nc.vector.tensor_copy(recip16[:], recip[:])
            for h in range(6 * half, 6 * half + 6):
                nt, r0 = h // 2, 64 * (h % 2)
                rep = np_.tile([64, 512], f16, tag="rep")
                q = nc.sync if h % 2 == 0 else nc.scalar
                q.dma_start(
                    rep[:],
                    recip16[h % 6:h % 6 + 1, None, :]
                    .broadcast_to((1, 64, 512)))
                dst = yT_t[nt][r0:r0 + D, bcol:bcol + T]
                if r0 == 0:
                    nc.vector.tensor_mul(dst, yun_all[(b, h)][:], rep[:])
                else:
                    st = np_.tile([64, 512], f16, tag="stage")
                    nc.vector.tensor_mul(st[:], yun_all[(b, h)][:], rep[:])
                    nc.gpsimd.dma_start(dst, st[:])

        pj_part = {}

        def proj_chunk(b, i, ks=0, ke=KT, partial=False):
            t = b * 4 + i // 2
            lo, w = ((0, 512), (512, 256))[i % 2]
            p = ps_mm.tile([128, 512], f32, tag="mm", name=f"pj{b}_{i}_{ks}")
            for k in range(ks, ke):
                nc.tensor.matmul(
                    p[:, :w],
                    yT_t[k][:, t * 128:(t + 1) * 128],
                    wp_t[k][:, lo:lo + w],
                    start=(k == ks), stop=(partial and k == ke - 1))
            if partial:
                pt = np_.tile([128, 512], f32, tag="pjpart", bufs=8,
                              name=f"pjpart{i}")
                nc.vector.tensor_copy(pt[:, :w], p[:, :w])
                pj_part[(b, i)] = pt
                return
            nc.tensor.matmul(
                p[:, :w],
                ones[0:1, t * 128:(t + 1) * 128],
                bp_t[0:1, lo:lo + w],
                start=False, stop=True)
            ot = np_.tile([128, 512], f32, tag="ostage", bufs=3)
            if (b, i) in pj_part:
                nc.vector.tensor_tensor(
                    out=ot[:, :w], in0=p[:, :w], in1=pj_part[(b, i)][:, :w],
                    op=mybir.AluOpType.add)
            else:
                nc.vector.tensor_copy(ot[:, :w], p[:, :w])
            nc.gpsimd.dma_start(
                out_d[t * 128:(t + 1) * 128, lo:lo + w], ot[:, :w])

        # software-pipelined emission
        for i in range(20):
            qkv_chain(0, i)
        qk1 = iter(range(20))
        for hp in range(6):
            attention_hp(0, hp)
            if hp == 3:
                norm_tail(0, 0)
            for _ in range(4 if hp < 2 else 3):
                i = next(qk1, None)
                if i is not None:
                    qkv_chain(1, i)
        norm_tail(0, 1)
        pj0 = iter(range(8))
        for hp in range(6):
            attention_hp(1, hp)
            if hp == 3:
                norm_tail(1, 0)
                for i in range(8):
                    proj_chunk(1, i, 0, KT // 2, partial=True)
            i = next(pj0, None)
            if i is not None:
                proj_chunk(0, i)
        norm_tail(1, 1)
        for i in pj0:
            proj_chunk(0, i)
        for i in range(8):
            proj_chunk(1, i, KT // 2, KT)

    nc.compile()
    return nc


def get_compiled():
    if "nc" not in _cache:
        _cache["nc"] = _build()
    return _cache["nc"]


def make_in_maps(x, attention_mask, W_attn, b_attn, W_proj, b_proj):
    x = np.asarray(x, dtype=np.float32).astype(np.float16)
    mask = np.ascontiguousarray(
        np.asarray(attention_mask, dtype=np.float32)[:, 0, 0, :])
    wa = np.asarray(W_attn, dtype=np.float32).astype(np.float16)
    ba = np.asarray(b_attn, dtype=np.float32).astype(np.float16).reshape(1, C3)
    wp = np.asarray(W_proj, dtype=np.float32).astype(np.float16)
    bp = np.asarray(b_proj, dtype=np.float32).astype(np.float16).reshape(1, C)
    maps = []
    for i in range(N_CORES):
        maps.append({
            "x": np.ascontiguousarray(x[BC * i:BC * (i + 1)].reshape(M, C)),
            "mask": np.ascontiguousarray(mask[BC * i:BC * (i + 1)]),
            "w_attn": wa, "b_attn": ba, "w_proj": wp, "b_proj": bp,
        })
    return maps


def kernel(x, attention_mask, W_attn, b_attn, W_proj, b_proj):
    from concourse.bass_utils import run_bass_kernel_spmd

    nc = get_compiled()
    in_maps = make_in_maps(x, attention_mask, W_attn, b_attn, W_proj, b_proj)
    last_err = None
    for _ in range(3):
        try:
            res = run_bass_kernel_spmd(nc, in_maps, list(range(N_CORES)))
            break
        except Exception as e:  # transient NRT device errors: retry
            last_err = e
    else:
        raise last_err
    out = np.concatenate(
        [res.results[i]["out"].reshape(BC, T, C) for i in range(N_CORES)], axis=0)
    return out.astype(np.float32)


# revision 25
# speedup vs baseline: 1.4648x; 1.0263x over previous
"""BERT self-attention (B=16, T=512, C=768, H=12, D=64) on 8 trn2 NeuronCores.

Data-parallel over batch: each core gets 2 batches. Matmul operands are fp16
(11-bit mantissa, ~tf32-class precision, 1 cycle/row PE streaming, FWL weight
loads); all accumulation stays fp32 in PSUM. Per core:
  xT    = x transposed during load via the DMA XBAR transpose (fp16).
  Q^T/K^T ([feature, token] layout, lhsT = W_attn tile) and V ([token, feature]
          layout with an interleaved ones column per head, lhsT = xT tile).
  S^T   = K^T-as-lhsT matmul -> scores in [key, query] layout (K=64, head pairs
          packed in PE row groups via base-partition-64 slices).
  P     = exp(S/8 + mask) on ScalarE (mask is a per-partition bias in this
          layout), written as fp16.
  y^T   = lhsT=[V_h | ones] matmul -> unnormalized y^T plus softmax row-sums as
          an extra PSUM row; row-sums are collected per batch, inverted in one
          batched DVE reciprocal, replicated across partitions by a
          broadcast-AP DMA, and applied with a DVE multiply.
  out   = y^T-as-lhsT matmul with W_proj + b_proj (fp32 result to DRAM).
Biases are folded in as K=1 accumulating matmuls against a ones row.
"""

import sys

sys.path.insert(0, "/opt/trn_rl_repo")

from contextlib import ExitStack

import numpy as np

B, T, C = 16, 512, 768
H, D = 12, 64
C3 = 3 * C
N_CORES = 8
BC = B // N_CORES           # batches per core
M = BC * T                  # tokens per core
KT = C // 128               # feature k-tiles (6)
TT = M // 128               # token tiles per core (8)
NQK = 2 * C // 128          # q+k feature n-tiles (12)
VW = H * (D + 1)            # v tile width with interleaved ones cols (780)
SCALE = 1.0 / np.sqrt(D)

_cache = {}


def _build():
    import concourse.bass as bass
    import concourse.tile as tile
    from concourse import bacc, mybir
    from concourse.masks import make_identity
    f32 = mybir.dt.float32
    f16 = mybir.dt.float16
    Exp = mybir.ActivationFunctionType.Exp

    nc = bacc.Bacc("TRN2", target_bir_lowering=False, debug=False,
                   num_devices=N_CORES)
    x_d = nc.dram_tensor("x", [M, C], f16, kind="ExternalInput").ap()
    mask_d = nc.dram_tensor("mask", [BC, T], f32, kind="ExternalInput").ap()
    wa_d = nc.dram_tensor("w_attn", [C, C3], f16, kind="ExternalInput").ap()
    ba_d = nc.dram_tensor("b_attn", [1, C3], f16, kind="ExternalInput").ap()
    wp_d = nc.dram_tensor("w_proj", [C, C], f16, kind="ExternalInput").ap()
    bp_d = nc.dram_tensor("b_proj", [1, C], f16, kind="ExternalInput").ap()
    out_d = nc.dram_tensor("out", [M, C], f32, kind="ExternalOutput").ap()

    with tile.TileContext(nc) as tc, ExitStack() as ctx:
        pp = ctx.enter_context(tc.tile_pool(name="pp", bufs=1))
        np_ = ctx.enter_context(tc.tile_pool(name="norm", bufs=4))
        ap_ = ctx.enter_context(tc.tile_pool(name="att", bufs=6))
        ps_mm = ctx.enter_context(tc.tile_pool(name="ps_mm", bufs=2, space="PSUM"))

        ones = pp.tile([1, M], f16, tag="ones")
        nc.vector.memset(ones[:], 1.0)
        mask_sb = pp.tile([128, BC * 4], f32, tag="mask")
        nc.gpsimd.dma_start(
            mask_sb[:],
            mask_d.rearrange("a b -> (a b)").rearrange("(j p) -> p j", p=128))
        ba_t = pp.tile([1, C3], f16, tag="ba")
        nc.gpsimd.dma_start(ba_t[:], ba_d[:])
        bp_t = pp.tile([1, C], f16, tag="bp")
        nc.gpsimd.dma_start(bp_t[:], bp_d[:])

        wa_t = [pp.tile([128, C3], f16, tag=f"wa{k}", name=f"wa{k}")
                for k in range(KT)]
        xT = [pp.tile([128, M], f16, tag=f"xT{k}", name=f"xT{k}")
              for k in range(KT)]
        ident = pp.tile([128, 128], f16, tag="ident")
        make_identity(nc, ident[:])
        with tc.tile_pool(name="ps_tr", bufs=4, space="PSUM") as ps_tr, \
                tc.tile_pool(name="xin", bufs=4) as xin:
            for t in range(TT):
                xt_in = xin.tile([128, C], f16, tag="x_in")
                q = nc.sync if t % 2 == 0 else nc.scalar
                q.dma_start(xt_in[:], x_d[t * 128:(t + 1) * 128, :])
                if t == 1:
                    for j in range(2):
                        for k in range(KT):
                            qw = nc.scalar if k % 2 == 0 else nc.sync
                            qw.dma_start(
                                wa_t[k][:, j * 1152:(j + 1) * 1152],
                                wa_d[k * 128:(k + 1) * 128,
                                     j * 1152:(j + 1) * 1152])
                for k in range(KT):
                    ptr = ps_tr.tile([128, 128], f16)
                    nc.tensor.transpose(
                        ptr[:], xt_in[:, k * 128:(k + 1) * 128], ident[:])
                    nc.vector.tensor_copy(
                        xT[k][:, t * 128:(t + 1) * 128], ptr[:])
        ps_s = ctx.enter_context(tc.tile_pool(name="ps_s", bufs=2, space="PSUM"))
        ps_y = ctx.enter_context(tc.tile_pool(name="ps_y", bufs=2, space="PSUM"))
        wp_t = [pp.tile([128, C], f16, tag=f"wp{k}", name=f"wp{k}")
                for k in range(KT)]
        for k in range(KT):
            nc.gpsimd.dma_start(wp_t[k][:], wp_d[k * 128:(k + 1) * 128, :])

        v_t = [pp.tile([128, VW], f16, tag=f"v{t}", name=f"v{t}")
               for t in range(TT)]
        qkT = [pp.tile([128, M], f16, tag=f"qk{n}", name=f"qk{n}")
               for n in range(NQK)]
        yT_t = [pp.tile([128, M], f16, tag=f"yT{c}", name=f"yT{c}")
                for c in range(KT)]
        for t in range(TT):
            nc.vector.memset(
                v_t[t].rearrange("p (h c) -> p h c", c=D + 1)
                    [:, :, D:D + 1], 1.0)

        def qkv_chain(b, i):
            """i in [0, 20): 12 QK n-tiles then 8 V half-tiles."""
            bcol = b * T
            if i < NQK:
                n = i
                p = ps_mm.tile([128, 512], f32, tag="mm", name=f"mm{b}_{i}")
                for k in range(KT):
                    nc.tensor.matmul(
                        p[:],
                        wa_t[k][:, n * 128:(n + 1) * 128],
                        xT[k][:, bcol:bcol + T],
                        start=(k == 0), stop=False)
                nc.tensor.matmul(
                    p[:],
                    ba_t[0:1, n * 128:(n + 1) * 128],
                    ones[0:1, bcol:bcol + T],
                    start=False, stop=True)
                nc.vector.tensor_copy(qkT[n][:, bcol:bcol + T], p[:])
            else:
                j = i - NQK
                t = b * 4 + j // 2
                lo, w = ((0, 512), (512, 256))[j % 2]
                p = ps_mm.tile([128, 512], f32, tag="mm", name=f"mm{b}_{i}")
                for k in range(KT):
                    nc.tensor.matmul(
                        p[:, :w],
                        xT[k][:, t * 128:(t + 1) * 128],
                        wa_t[k][:, 2 * C + lo:2 * C + lo + w],
                        start=(k == 0), stop=False)
                nc.tensor.matmul(
                    p[:, :w],
                    ones[0:1, t * 128:(t + 1) * 128],
                    ba_t[0:1, 2 * C + lo:2 * C + lo + w],
                    start=False, stop=True)
                h0 = lo // D
                nc.vector.tensor_copy(
                    v_t[t].rearrange("p (h c) -> p h c", c=D + 1)
                        [:, h0:h0 + w // D, 0:D],
                    p[:, :w].rearrange("p (h c) -> p h c", c=D))

        yun_all = {}
        r_tiles = {}

        def attention_hp(b, hp):
            bcol = b * T
            if hp % 3 == 0:
                r_tiles[(b, hp // 3)] = np_.tile(
                    [H // 2, 512], f32, tag="r_all", bufs=4,
                    name=f"r_all{b}_{hp // 3}")
            e_tiles = []
            for kt in range(4):
                ps = ps_s.tile([128, 1024], f32)
                for sub in range(2):
                    r0 = 64 * sub
                    nc.tensor.matmul(
                        ps[:, sub * 512:sub * 512 + 512],
                        qkT[6 + hp][r0:r0 + D,
                                    bcol + kt * 128:bcol + (kt + 1) * 128],
                        qkT[hp][r0:r0 + D, bcol:bcol + T],
                        start=True, stop=True)
                e = ap_.tile([128, 1024], f16, tag="e")
                nc.scalar.activation(
                    e[:], ps[:], Exp,
                    bias=mask_sb[:, b * 4 + kt:b * 4 + kt + 1],
                    scale=float(SCALE))
                e_tiles.append(e)
            for sub in range(2):
                h = 2 * hp + sub
                py = ps_y.tile([128, 512], f32)
                for kt in range(4):
                    nc.tensor.matmul(
                        py[0:D + 1, :],
                        v_t[b * 4 + kt][:, (D + 1) * h:(D + 1) * (h + 1)],
                        e_tiles[kt][:, sub * 512:sub * 512 + 512],
                        start=(kt == 0), stop=(kt == 3))
                yun = np_.tile([64, 512], f16, tag="yun", bufs=14,
                               name=f"yun{b}_{h}")
                nc.vector.tensor_copy(yun[:], py[0:D, :])
                rs = np_.tile([D + 1, 512], f32, tag="rstage")
                nc.scalar.copy(rs[D:D + 1, :], py[D:D + 1, :])
                nc.sync.dma_start(
                    r_tiles[(b, hp // 3)][h % 6:h % 6 + 1, :], rs[D:D + 1, :])
                yun_all[(b, h)] = yun

        def norm_tail(b, half):
            bcol = b * T
            recip = np_.tile([H // 2, 512], f32, tag="recip")
            nc.vector.reciprocal(recip[:], r_tiles[(b, half)][:])
            recip16 = np_.tile([H // 2, 512], f16, tag="recip16")
            nc.vector.tensor_copy(recip16[:], recip[:])
            for h in range(6 * half, 6 * half + 6):
                nt, r0 = h // 2, 64 * (h % 2)
                rep = np_.tile([64, 512], f16, tag="rep")
                q = nc.sync if h % 2 == 0 else nc.scalar
                q.dma_start(
                    rep[:],
                    recip16[h % 6:h % 6 + 1, None, :]
                    .broadcast_to((1, 64, 512)))
                dst = yT_t[nt][r0:r0 + D, bcol:bcol + T]
                if r0 == 0:
                    nc.vector.tensor_mul(dst, yun_all[(b, h)][:], rep[:])
                else:
                    st = np_.tile([64, 512], f16, tag="stage")
                    nc.vector.tensor_mul(st[:], yun_all[(b, h)][:], rep[:])
                    nc.gpsimd.dma_start(dst, st[:])

        pj_part = {}

        def proj_chunk(b, i, ks=0, ke=KT, partial=False):
            t = b * 4 + i // 2
            lo, w = ((0, 512), (512, 256))[i % 2]
            p = ps_mm.tile([128, 512], f32, tag="mm", name=f"pj{b}_{i}_{ks}")
            for k in range(ks, ke):
                nc.tensor.matmul(
                    p[:, :w],
                    yT_t[k][:, t * 128:(t + 1) * 128],
                    wp_t[k][:, lo:lo + w],
                    start=(k == ks), stop=(partial and k == ke - 1))
            if partial:
                pt = np_.tile([128, 512], f32, tag="pjpart", bufs=8,
                              name=f"pjpart{i}")
                nc.vector.tensor_copy(pt[:, :w], p[:, :w])
                pj_part[(b, i)] = pt
                return
            nc.tensor.matmul(
                p[:, :w],
                ones[0:1, t * 128:(t + 1) * 128],
                bp_t[0:1, lo:lo + w],
                start=False, stop=True)
            ot = np_.tile([128, 512], f32, tag="ostage", bufs=3)
            if (b, i) in pj_part:
                nc.vector.tensor_tensor(
                    out=ot[:, :w], in0=p[:, :w], in1=pj_part[(b, i)][:, :w],
                    op=mybir.AluOpType.add)
            else:
                nc.vector.tensor_copy(ot[:, :w], p[:, :w])
            nc.gpsimd.dma_start(
                out_d[t * 128:(t + 1) * 128, lo:lo + w], ot[:, :w])

        # software-pipelined emission
        for i in range(20):
            qkv_chain(0, i)
        qk1 = iter(range(20))
        for hp in range(6):
            attention_hp(0, hp)
            if hp == 3:
                norm_tail(0, 0)
            for _ in range(4 if hp < 2 else 3):
                i = next(qk1, None)
                if i is not None:
                    qkv_chain(1, i)
        norm_tail(0, 1)
        pj0 = iter(range(8))
        for hp in range(6):
            attention_hp(1, hp)
            if hp == 3:
                norm_tail(1, 0)
                for i in range(8):
                    proj_chunk(1, i, 0, KT // 2, partial=True)
            i = next(pj0, None)
            if i is not None:
                proj_chunk(0, i)
        norm_tail(1, 1)
        for i in pj0:
            proj_chunk(0, i)
        for i in range(8):
            proj_chunk(1, i, KT // 2, KT)

    nc.compile()
    return nc


def get_compiled():
    if "nc" not in _cache:
        _cache["nc"] = _build()
    return _cache["nc"]


def make_in_maps(x, attention_mask, W_attn, b_attn, W_proj, b_proj):
    x = np.asarray(x, dtype=np.float32).astype(np.float16)
    mask = np.ascontiguousarray(
        np.asarray(attention_mask, dtype=np.float32)[:, 0, 0, :])
    wa = np.asarray(W_attn, dtype=np.float32).astype(np.float16)
    ba = np.asarray(b_attn, dtype=np.float32).astype(np.float16).reshape(1, C3)
    wp = np.asarray(W_proj, dtype=np.float32).astype(np.float16)
    bp = np.asarray(b_proj, dtype=np.float32).astype(np.float16).reshape(1, C)
    maps = []
    for i in range(N_CORES):
        maps.append({
            "x": np.ascontiguousarray(x[BC * i:BC * (i + 1)].reshape(M, C)),
            "mask": np.ascontiguousarray(mask[BC * i:BC * (i + 1)]),
            "w_attn": wa, "b_attn": ba, "w_proj": wp, "b_proj": bp,
        })
    return maps


def kernel(x, attention_mask, W_attn, b_attn, W_proj, b_proj):
    from concourse.bass_utils import run_bass_kernel_spmd

    nc = get_compiled()
    in_maps = make_in_maps(x, attention_mask, W_attn, b_attn, W_proj, b_proj)
    last_err = None
    for _ in range(3):
        try:
            res = run_bass_kernel_spmd(nc, in_maps, list(range(N_CORES)))
            break
        except Exception as e:  # transient NRT device errors: retry
            last_err = e
    else:
        raise last_err
    out = np.concatenate(
        [res.results[i]["out"].reshape(BC, T, C) for i in range(N_CORES)], axis=0)
    return out.astype(np.float32)


# revision 26
# speedup vs baseline: 1.5032x; 1.0262x over previous
"""BERT self-attention (B=16, T=512, C=768, H=12, D=64) on 8 trn2 NeuronCores.

Data-parallel over batch: each core gets 2 batches. Matmul operands are fp16
(11-bit mantissa, ~tf32-class precision, 1 cycle/row PE streaming, FWL weight
loads); all accumulation stays fp32 in PSUM. Per core:
  xT    = x transposed during load via the DMA XBAR transpose (fp16).
  Q^T/K^T ([feature, token] layout, lhsT = W_attn tile) and V ([token, feature]
          layout with an interleaved ones column per head, lhsT = xT tile).
  S^T   = K^T-as-lhsT matmul -> scores in [key, query] layout (K=64, head pairs
          packed in PE row groups via base-partition-64 slices).
  P     = exp(S/8 + mask) on ScalarE (mask is a per-partition bias in this
          layout), written as fp16.
  y^T   = lhsT=[V_h | ones] matmul -> unnormalized y^T plus softmax row-sums as
          an extra PSUM row; row-sums are collected per batch, inverted in one
          batched DVE reciprocal, replicated across partitions by a
          broadcast-AP DMA, and applied with a DVE multiply.
  out   = y^T-as-lhsT matmul with W_proj + b_proj (fp32 result to DRAM).
Biases are folded in as K=1 accumulating matmuls against a ones row.
"""

import sys

sys.path.insert(0, "/opt/trn_rl_repo")

from contextlib import ExitStack

import numpy as np

B, T, C = 16, 512, 768
H, D = 12, 64
C3 = 3 * C
N_CORES = 8
BC = B // N_CORES           # batches per core
M = BC * T                  # tokens per core
KT = C // 128               # feature k-tiles (6)
TT = M // 128               # token tiles per core (8)
NQK = 2 * C // 128          # q+k feature n-tiles (12)
VW = H * (D + 1)            # v tile width with interleaved ones cols (780)
SCALE = 1.0 / np.sqrt(D)

_cache = {}


def _build():
    import concourse.bass as bass
    import concourse.tile as tile
    from concourse import bacc, mybir
    from concourse.masks import make_identity
    f32 = mybir.dt.float32
    f16 = mybir.dt.float16
    Exp = mybir.ActivationFunctionType.Exp

    nc = bacc.Bacc("TRN2", target_bir_lowering=False, debug=False,
                   num_devices=N_CORES)
    x_d = nc.dram_tensor("x", [M, C], f16, kind="ExternalInput").ap()
    mask_d = nc.dram_tensor("mask", [BC, T], f32, kind="ExternalInput").ap()
    wa_d = nc.dram_tensor("w_attn", [C, C3], f16, kind="ExternalInput").ap()
    ba_d = nc.dram_tensor("b_attn", [1, C3], f16, kind="ExternalInput").ap()
    wp_d = nc.dram_tensor("w_proj", [C, C], f16, kind="ExternalInput").ap()
    bp_d = nc.dram_tensor("b_proj", [1, C], f16, kind="ExternalInput").ap()
    out_d = nc.dram_tensor("out", [M, C], f32, kind="ExternalOutput").ap()

    with tile.TileContext(nc) as tc, ExitStack() as ctx:
        pp = ctx.enter_context(tc.tile_pool(name="pp", bufs=1))
        np_ = ctx.enter_context(tc.tile_pool(name="norm", bufs=4))
        ap_ = ctx.enter_context(tc.tile_pool(name="att", bufs=6))
        ps_mm = ctx.enter_context(tc.tile_pool(name="ps_mm", bufs=2, space="PSUM"))

        ones = pp.tile([1, M], f16, tag="ones")
        nc.vector.memset(ones[:], 1.0)
        mask_sb = pp.tile([128, BC * 4], f32, tag="mask")
        nc.gpsimd.dma_start(
            mask_sb[:],
            mask_d.rearrange("a b -> (a b)").rearrange("(j p) -> p j", p=128))
        ba_t = pp.tile([1, C3], f16, tag="ba")
        nc.gpsimd.dma_start(ba_t[:], ba_d[:])
        bp_t = pp.tile([1, C], f16, tag="bp")
        nc.gpsimd.dma_start(bp_t[:], bp_d[:])

        wa_t = [pp.tile([128, C3], f16, tag=f"wa{k}", name=f"wa{k}")
                for k in range(KT)]
        xT = [pp.tile([128, M], f16, tag=f"xT{k}", name=f"xT{k}")
              for k in range(KT)]
        ident = pp.tile([128, 128], f16, tag="ident")
        make_identity(nc, ident[:])
        with tc.tile_pool(name="ps_tr", bufs=4, space="PSUM") as ps_tr, \
                tc.tile_pool(name="xin", bufs=4) as xin:
            xt_ins = []
            for t in range(TT):
                xt_in = xin.tile([128, C], f16, tag="x_in", bufs=8,
                                 name=f"x_in{t}")
                q = nc.sync if t % 2 == 0 else nc.scalar
                q.dma_start(xt_in[:], x_d[t * 128:(t + 1) * 128, :])
                xt_ins.append(xt_in)
            for j in range(2):
                for k in range(KT):
                    qw = nc.scalar if k % 2 == 0 else nc.sync
                    qw.dma_start(
                        wa_t[k][:, j * 1152:(j + 1) * 1152],
                        wa_d[k * 128:(k + 1) * 128, j * 1152:(j + 1) * 1152])
            for t in range(TT):
                for k in range(KT):
                    ptr = ps_tr.tile([128, 128], f16)
                    nc.tensor.transpose(
                        ptr[:], xt_ins[t][:, k * 128:(k + 1) * 128], ident[:])
                    nc.vector.tensor_copy(
                        xT[k][:, t * 128:(t + 1) * 128], ptr[:])
        ps_s = ctx.enter_context(tc.tile_pool(name="ps_s", bufs=2, space="PSUM"))
        ps_y = ctx.enter_context(tc.tile_pool(name="ps_y", bufs=2, space="PSUM"))
        wp_t = [pp.tile([128, C], f16, tag=f"wp{k}", name=f"wp{k}")
                for k in range(KT)]
        for k in range(KT):
            nc.gpsimd.dma_start(wp_t[k][:], wp_d[k * 128:(k + 1) * 128, :])

        v_t = [pp.tile([128, VW], f16, tag=f"v{t}", name=f"v{t}")
               for t in range(TT)]
        qkT = [pp.tile([128, M], f16, tag=f"qk{n}", name=f"qk{n}")
               for n in range(NQK)]
        yT_t = [pp.tile([128, M], f16, tag=f"yT{c}", name=f"yT{c}")
                for c in range(KT)]
        for t in range(TT):
            nc.vector.memset(
                v_t[t].rearrange("p (h c) -> p h c", c=D + 1)
                    [:, :, D:D + 1], 1.0)

        def qkv_chain(b, i):
            """i in [0, 20): 12 QK n-tiles then 8 V half-tiles."""
            bcol = b * T
            if i < NQK:
                n = i
                p = ps_mm.tile([128, 512], f32, tag="mm", name=f"mm{b}_{i}")
                for k in range(KT):
                    nc.tensor.matmul(
                        p[:],
                        wa_t[k][:, n * 128:(n + 1) * 128],
                        xT[k][:, bcol:bcol + T],
                        start=(k == 0), stop=False)
                nc.tensor.matmul(
                    p[:],
                    ba_t[0:1, n * 128:(n + 1) * 128],
                    ones[0:1, bcol:bcol + T],
                    start=False, stop=True)
                nc.vector.tensor_copy(qkT[n][:, bcol:bcol + T], p[:])
            else:
                j = i - NQK
                t = b * 4 + j // 2
                lo, w = ((0, 512), (512, 256))[j % 2]
                p = ps_mm.tile([128, 512], f32, tag="mm", name=f"mm{b}_{i}")
                for k in range(KT):
                    nc.tensor.matmul(
                        p[:, :w],
                        xT[k][:, t * 128:(t + 1) * 128],
                        wa_t[k][:, 2 * C + lo:2 * C + lo + w],
                        start=(k == 0), stop=False)
                nc.tensor.matmul(
                    p[:, :w],
                    ones[0:1, t * 128:(t + 1) * 128],
                    ba_t[0:1, 2 * C + lo:2 * C + lo + w],
                    start=False, stop=True)
                h0 = lo // D
                nc.vector.tensor_copy(
                    v_t[t].rearrange("p (h c) -> p h c", c=D + 1)
                        [:, h0:h0 + w // D, 0:D],
                    p[:, :w].rearrange("p (h c) -> p h c", c=D))

        yun_all = {}
        r_tiles = {}

        def attention_hp(b, hp):
            bcol = b * T
            if hp % 3 == 0:
                r_tiles[(b, hp // 3)] = np_.tile(
                    [H // 2, 512], f32, tag="r_all", bufs=4,
                    name=f"r_all{b}_{hp // 3}")
            e_tiles = []
            for kt in range(4):
                ps = ps_s.tile([128, 1024], f32)
                for sub in range(2):
                    r0 = 64 * sub
                    nc.tensor.matmul(
                        ps[:, sub * 512:sub * 512 + 512],
                        qkT[6 + hp][r0:r0 + D,
                                    bcol + kt * 128:bcol + (kt + 1) * 128],
                        qkT[hp][r0:r0 + D, bcol:bcol + T],
                        start=True, stop=True)
                e = ap_.tile([128, 1024], f16, tag="e")
                nc.scalar.activation(
                    e[:], ps[:], Exp,
                    bias=mask_sb[:, b * 4 + kt:b * 4 + kt + 1],
                    scale=float(SCALE))
                e_tiles.append(e)
            for sub in range(2):
                h = 2 * hp + sub
                py = ps_y.tile([128, 512], f32)
                for kt in range(4):
                    nc.tensor.matmul(
                        py[0:D + 1, :],
                        v_t[b * 4 + kt][:, (D + 1) * h:(D + 1) * (h + 1)],
                        e_tiles[kt][:, sub * 512:sub * 512 + 512],
                        start=(kt == 0), stop=(kt == 3))
                yun = np_.tile([64, 512], f16, tag="yun", bufs=14,
                               name=f"yun{b}_{h}")
                nc.vector.tensor_copy(yun[:], py[0:D, :])
                rs = np_.tile([D + 1, 512], f32, tag="rstage")
                nc.scalar.copy(rs[D:D + 1, :], py[D:D + 1, :])
                nc.sync.dma_start(
                    r_tiles[(b, hp // 3)][h % 6:h % 6 + 1, :], rs[D:D + 1, :])
                yun_all[(b, h)] = yun

        def norm_tail(b, half):
            bcol = b * T
            recip = np_.tile([H // 2, 512], f32, tag="recip")
            nc.vector.reciprocal(recip[:], r_tiles[(b, half)][:])
            recip16 = np_.tile([H // 2, 512], f16, tag="recip16")
            nc.vector.tensor_copy(recip16[:], recip[:])
            for h in range(6 * half, 6 * half + 6):
                nt, r0 = h // 2, 64 * (h % 2)
                rep = np_.tile([64, 512], f16, tag="rep")
                q = nc.sync if h % 2 == 0 else nc.scalar
                q.dma_start(
                    rep[:],
                    recip16[h % 6:h % 6 + 1, None, :]
                    .broadcast_to((1, 64, 512)))
                dst = yT_t[nt][r0:r0 + D, bcol:bcol + T]
                if r0 == 0:
                    nc.vector.tensor_mul(dst, yun_all[(b, h)][:], rep[:])
                else:
                    st = np_.tile([64, 512], f16, tag="stage")
                    nc.vector.tensor_mul(st[:], yun_all[(b, h)][:], rep[:])
                    nc.gpsimd.dma_start(dst, st[:])

        pj_part = {}

        def proj_chunk(b, i, ks=0, ke=KT, partial=False):
            t = b * 4 + i // 2
            lo, w = ((0, 512), (512, 256))[i % 2]
            p = ps_mm.tile([128, 512], f32, tag="mm", name=f"pj{b}_{i}_{ks}")
            for k in range(ks, ke):
                nc.tensor.matmul(
                    p[:, :w],
                    yT_t[k][:, t * 128:(t + 1) * 128],
                    wp_t[k][:, lo:lo + w],
                    start=(k == ks), stop=(partial and k == ke - 1))
            if partial:
                pt = np_.tile([128, 512], f32, tag="pjpart", bufs=8,
                              name=f"pjpart{i}")
                nc.vector.tensor_copy(pt[:, :w], p[:, :w])
                pj_part[(b, i)] = pt
                return
            nc.tensor.matmul(
                p[:, :w],
                ones[0:1, t * 128:(t + 1) * 128],
                bp_t[0:1, lo:lo + w],
                start=False, stop=True)
            ot = np_.tile([128, 512], f32, tag="ostage", bufs=3)
            if (b, i) in pj_part:
                nc.vector.tensor_tensor(
                    out=ot[:, :w], in0=p[:, :w], in1=pj_part[(b, i)][:, :w],
                    op=mybir.AluOpType.add)
            else:
                nc.vector.tensor_copy(ot[:, :w], p[:, :w])
            nc.gpsimd.dma_start(
                out_d[t * 128:(t + 1) * 128, lo:lo + w], ot[:, :w])

        # software-pipelined emission
        for i in range(20):
            qkv_chain(0, i)
        qk1 = iter(range(20))
        for hp in range(6):
            attention_hp(0, hp)
            if hp == 3:
                norm_tail(0, 0)
            for _ in range(4 if hp < 2 else 3):
                i = next(qk1, None)
                if i is not None:
                    qkv_chain(1, i)
        norm_tail(0, 1)
        pj0 = iter(range(8))
        for hp in range(6):
            attention_hp(1, hp)
            if hp == 3:
                norm_tail(1, 0)
                for i in range(8):
                    proj_chunk(1, i, 0, KT // 2, partial=True)
            i = next(pj0, None)
            if i is not None:
                proj_chunk(0, i)
        norm_tail(1, 1)
        for i in pj0:
            proj_chunk(0, i)
        for i in range(8):
            proj_chunk(1, i, KT // 2, KT)

    nc.compile()
    return nc


def get_compiled():
    if "nc" not in _cache:
        _cache["nc"] = _build()
    return _cache["nc"]


def make_in_maps(x, attention_mask, W_attn, b_attn, W_proj, b_proj):
    x = np.asarray(x, dtype=np.float32).astype(np.float16)
    mask = np.ascontiguousarray(
        np.asarray(attention_mask, dtype=np.float32)[:, 0, 0, :])
    wa = np.asarray(W_attn, dtype=np.float32).astype(np.float16)
    ba = np.asarray(b_attn, dtype=np.float32).astype(np.float16).reshape(1, C3)
    wp = np.asarray(W_proj, dtype=np.float32).astype(np.float16)
    bp = np.asarray(b_proj, dtype=np.float32).astype(np.float16).reshape(1, C)
    maps = []
    for i in range(N_CORES):
        maps.append({
            "x": np.ascontiguousarray(x[BC * i:BC * (i + 1)].reshape(M, C)),
            "mask": np.ascontiguousarray(mask[BC * i:BC * (i + 1)]),
            "w_attn": wa, "b_attn": ba, "w_proj": wp, "b_proj": bp,
        })
    return maps


def kernel(x, attention_mask, W_attn, b_attn, W_proj, b_proj):
    from concourse.bass_utils import run_bass_kernel_spmd

    nc = get_compiled()
    in_maps = make_in_maps(x, attention_mask, W_attn, b_attn, W_proj, b_proj)
    last_err = None
    for _ in range(3):
        try:
            res = run_bass_kernel_spmd(nc, in_maps, list(range(N_CORES)))
            break
        except Exception as e:  # transient NRT device errors: retry
            last_err = e
    else:
        raise last_err
    out = np.concatenate(
        [res.results[i]["out"].reshape(BC, T, C) for i in range(N_CORES)], axis=0)
    return out.astype(np.float32)


# revision 29
# speedup vs baseline: 1.5314x; 1.0188x over previous
"""BERT self-attention (B=16, T=512, C=768, H=12, D=64) on 8 trn2 NeuronCores.

Data-parallel over batch: each core gets 2 batches. Matmul operands are fp16
(11-bit mantissa, ~tf32-class precision, 1 cycle/row PE streaming, FWL weight
loads); all accumulation stays fp32 in PSUM. Per core:
  xT    = x transposed during load via the DMA XBAR transpose (fp16).
  Q^T/K^T ([feature, token] layout, lhsT = W_attn tile) and V ([token, feature]
          layout with an interleaved ones column per head, lhsT = xT tile).
  S^T   = K^T-as-lhsT matmul -> scores in [key, query] layout (K=64, head pairs
          packed in PE row groups via base-partition-64 slices).
  P     = exp(S/8 + mask) on ScalarE (mask is a per-partition bias in this
          layout), written as fp16.
  y^T   = lhsT=[V_h | ones] matmul -> unnormalized y^T plus softmax row-sums as
          an extra PSUM row; row-sums are collected per batch, inverted in one
          batched DVE reciprocal, replicated across partitions by a
          broadcast-AP DMA, and applied with a DVE multiply.
  out   = y^T-as-lhsT matmul with W_proj + b_proj (fp32 result to DRAM).
Biases are folded in as K=1 accumulating matmuls against a ones row.
"""

import sys

sys.path.insert(0, "/opt/trn_rl_repo")

from contextlib import ExitStack

import numpy as np

B, T, C = 16, 512, 768
H, D = 12, 64
C3 = 3 * C
N_CORES = 8
BC = B // N_CORES           # batches per core
M = BC * T                  # tokens per core
KT = C // 128               # feature k-tiles (6)
TT = M // 128               # token tiles per core (8)
NQK = 2 * C // 128          # q+k feature n-tiles (12)
VW = H * (D + 1)            # v tile width with interleaved ones cols (780)
SCALE = 1.0 / np.sqrt(D)

_cache = {}


def _build():
    import concourse.bass as bass
    import concourse.tile as tile
    from concourse import bacc, mybir
    from concourse.masks import make_identity
    f32 = mybir.dt.float32
    f16 = mybir.dt.float16
    Exp = mybir.ActivationFunctionType.Exp

    nc = bacc.Bacc("TRN2", target_bir_lowering=False, debug=False,
                   num_devices=N_CORES)
    x_d = nc.dram_tensor("x", [M, C], f16, kind="ExternalInput").ap()
    mask_d = nc.dram_tensor("mask", [BC, T], f32, kind="ExternalInput").ap()
    wa_d = nc.dram_tensor("w_attn", [C, C3], f16, kind="ExternalInput").ap()
    ba_d = nc.dram_tensor("b_attn", [1, C3], f16, kind="ExternalInput").ap()
    wp_d = nc.dram_tensor("w_proj", [C, C], f16, kind="ExternalInput").ap()
    bp_d = nc.dram_tensor("b_proj", [1, C], f16, kind="ExternalInput").ap()
    out_d = nc.dram_tensor("out", [M, C], f32, kind="ExternalOutput").ap()

    with tile.TileContext(nc) as tc, ExitStack() as ctx:
        pp = ctx.enter_context(tc.tile_pool(name="pp", bufs=1))
        np_ = ctx.enter_context(tc.tile_pool(name="norm", bufs=4))
        ap_ = ctx.enter_context(tc.tile_pool(name="att", bufs=6))
        ps_mm = ctx.enter_context(tc.tile_pool(name="ps_mm", bufs=2, space="PSUM"))

        ones = pp.tile([1, M], f16, tag="ones")
        nc.vector.memset(ones[:], 1.0)
        mask_sb = pp.tile([128, BC * 4], f32, tag="mask")
        nc.gpsimd.dma_start(
            mask_sb[:],
            mask_d.rearrange("a b -> (a b)").rearrange("(j p) -> p j", p=128))
        ba_t = pp.tile([1, C3], f16, tag="ba")
        nc.gpsimd.dma_start(ba_t[:], ba_d[:])
        bp_t = pp.tile([1, C], f16, tag="bp")
        nc.gpsimd.dma_start(bp_t[:], bp_d[:])

        wa_t = [pp.tile([128, C3], f16, tag=f"wa{k}", name=f"wa{k}")
                for k in range(KT)]
        xT = [pp.tile([128, M], f16, tag=f"xT{k}", name=f"xT{k}")
              for k in range(KT)]
        ident = pp.tile([128, 128], f16, tag="ident")
        make_identity(nc, ident[:])
        ones32 = pp.tile([128, 64], f32, tag="ones32")
        nc.vector.memset(ones32[:], 1.0)
        with tc.tile_pool(name="ps_tr", bufs=4, space="PSUM") as ps_tr, \
                tc.tile_pool(name="xin", bufs=4) as xin:
            xt_ins = []
            for t in range(TT):
                xt_in = xin.tile([128, C], f16, tag="x_in", bufs=8,
                                 name=f"x_in{t}")
                q = nc.sync if t % 2 == 0 else nc.scalar
                q.dma_start(xt_in[:], x_d[t * 128:(t + 1) * 128, :])
                xt_ins.append(xt_in)
            for j in range(2):
                for k in range(KT):
                    qw = nc.scalar if k % 2 == 0 else nc.sync
                    qw.dma_start(
                        wa_t[k][:, j * 1152:(j + 1) * 1152],
                        wa_d[k * 128:(k + 1) * 128, j * 1152:(j + 1) * 1152])
            for t in range(TT):
                for k in range(KT):
                    ptr = ps_tr.tile([128, 128], f16)
                    nc.tensor.transpose(
                        ptr[:], xt_ins[t][:, k * 128:(k + 1) * 128], ident[:])
                    nc.vector.tensor_copy(
                        xT[k][:, t * 128:(t + 1) * 128], ptr[:])
        ps_s = ctx.enter_context(tc.tile_pool(name="ps_s", bufs=2, space="PSUM"))
        ps_y = ctx.enter_context(tc.tile_pool(name="ps_y", bufs=2, space="PSUM"))
        wp_t = [pp.tile([128, C], f16, tag=f"wp{k}", name=f"wp{k}")
                for k in range(KT)]
        for k in range(KT):
            nc.gpsimd.dma_start(wp_t[k][:], wp_d[k * 128:(k + 1) * 128, :])

        v_t = [pp.tile([128, VW], f16, tag=f"v{t}", name=f"v{t}")
               for t in range(TT)]
        qkT = [pp.tile([128, M], f16, tag=f"qk{n}", name=f"qk{n}")
               for n in range(NQK)]
        yT_t = [pp.tile([128, M], f16, tag=f"yT{c}", name=f"yT{c}")
                for c in range(KT)]
        for t in range(TT):
            nc.vector.memset(
                v_t[t].rearrange("p (h c) -> p h c", c=D + 1)
                    [:, :, D:D + 1], 1.0)

        def qkv_chain(b, i):
            """i in [0, 20): 12 QK n-tiles then 8 V half-tiles."""
            bcol = b * T
            if i < NQK:
                n = i
                p = ps_mm.tile([128, 512], f32, tag="mm", name=f"mm{b}_{i}")
                for k in range(KT):
                    nc.tensor.matmul(
                        p[:],
                        wa_t[k][:, n * 128:(n + 1) * 128],
                        xT[k][:, bcol:bcol + T],
                        start=(k == 0), stop=False)
                nc.tensor.matmul(
                    p[:],
                    ba_t[0:1, n * 128:(n + 1) * 128],
                    ones[0:1, bcol:bcol + T],
                    start=False, stop=True)
                nc.vector.tensor_copy(qkT[n][:, bcol:bcol + T], p[:])
            else:
                j = i - NQK
                t = b * 4 + j // 2
                lo, w = ((0, 512), (512, 256))[j % 2]
                p = ps_mm.tile([128, 512], f32, tag="mm", name=f"mm{b}_{i}")
                for k in range(KT):
                    nc.tensor.matmul(
                        p[:, :w],
                        xT[k][:, t * 128:(t + 1) * 128],
                        wa_t[k][:, 2 * C + lo:2 * C + lo + w],
                        start=(k == 0), stop=False)
                nc.tensor.matmul(
                    p[:, :w],
                    ones[0:1, t * 128:(t + 1) * 128],
                    ba_t[0:1, 2 * C + lo:2 * C + lo + w],
                    start=False, stop=True)
                h0 = lo // D
                nc.vector.tensor_copy(
                    v_t[t].rearrange("p (h c) -> p h c", c=D + 1)
                        [:, h0:h0 + w // D, 0:D],
                    p[:, :w].rearrange("p (h c) -> p h c", c=D))

        yun_all = {}
        r_tiles = {}

        def attention_hp(b, hp):
            bcol = b * T
            if hp % 2 == 0:
                rt = np_.tile([97, 512], f32, tag="r_all", bufs=3,
                              name=f"r_all{b}_{hp // 2}")
                nc.vector.memset(rt[:], 1.0)
                r_tiles[(b, hp // 2)] = rt
            e_tiles = []
            for kt in range(4):
                ps = ps_s.tile([128, 1024], f32)
                for sub in range(2):
                    r0 = 64 * sub
                    nc.tensor.matmul(
                        ps[:, sub * 512:sub * 512 + 512],
                        qkT[6 + hp][r0:r0 + D,
                                    bcol + kt * 128:bcol + (kt + 1) * 128],
                        qkT[hp][r0:r0 + D, bcol:bcol + T],
                        start=True, stop=True)
                e = ap_.tile([128, 1024], f16, tag="e")
                nc.scalar.activation(
                    e[:], ps[:], Exp,
                    bias=mask_sb[:, b * 4 + kt:b * 4 + kt + 1],
                    scale=float(SCALE))
                e_tiles.append(e)
            for sub in range(2):
                h = 2 * hp + sub
                py = ps_y.tile([128, 512], f32)
                for kt in range(4):
                    nc.tensor.matmul(
                        py[0:D + 1, :],
                        v_t[b * 4 + kt][:, (D + 1) * h:(D + 1) * (h + 1)],
                        e_tiles[kt][:, sub * 512:sub * 512 + 512],
                        start=(kt == 0), stop=(kt == 3))
                yun = np_.tile([64, 512], f16, tag="yun", bufs=14,
                               name=f"yun{b}_{h}")
                nc.vector.tensor_copy(yun[:], py[0:D, :])
                rs = np_.tile([D + 1, 512], f32, tag="rstage")
                nc.scalar.copy(rs[D:D + 1, :], py[D:D + 1, :])
                nc.sync.dma_start(
                    r_tiles[(b, hp // 2)][32 * (h % 4):32 * (h % 4) + 1, :],
                    rs[D:D + 1, :])
                yun_all[(b, h)] = yun

        def norm_tail(b, grp):
            bcol = b * T
            recip = np_.tile([97, 512], f32, tag="recip", bufs=3)
            nc.vector.reciprocal(recip[:], r_tiles[(b, grp)][:])
            for h in range(4 * grp, 4 * grp + 4):
                nt, r0 = h // 2, 64 * (h % 2)
                j = 32 * (h % 4)
                rep = ps_y.tile([128, 512], f32, tag="py", name=f"rep{b}_{h}")
                nc.tensor.matmul(
                    rep[0:64, :], ones32[j:j + 1, :], recip[j:j + 1, :],
                    start=True, stop=True, tile_position=(j, 0))
                dst = yT_t[nt][r0:r0 + D, bcol:bcol + T]
                if r0 == 0:
                    nc.vector.tensor_mul(dst, yun_all[(b, h)][:], rep[0:64, :])
                else:
                    st = np_.tile([64, 512], f16, tag="stage")
                    nc.vector.tensor_mul(st[:], yun_all[(b, h)][:],
                                         rep[0:64, :])
                    nc.gpsimd.dma_start(dst, st[:])

        pj_part = {}

        def proj_chunk(b, i, ks=0, ke=KT, partial=False):
            t = b * 4 + i // 2
            lo, w = ((0, 512), (512, 256))[i % 2]
            p = ps_mm.tile([128, 512], f32, tag="mm", name=f"pj{b}_{i}_{ks}")
            for k in range(ks, ke):
                nc.tensor.matmul(
                    p[:, :w],
                    yT_t[k][:, t * 128:(t + 1) * 128],
                    wp_t[k][:, lo:lo + w],
                    start=(k == ks), stop=(partial and k == ke - 1))
            if partial:
                pt = np_.tile([128, 512], f32, tag="pjpart", bufs=8,
                              name=f"pjpart{i}")
                nc.vector.tensor_copy(pt[:, :w], p[:, :w])
                pj_part[(b, i)] = pt
                return
            nc.tensor.matmul(
                p[:, :w],
                ones[0:1, t * 128:(t + 1) * 128],
                bp_t[0:1, lo:lo + w],
                start=False, stop=True)
            ot = np_.tile([128, 512], f32, tag="ostage", bufs=3)
            if (b, i) in pj_part:
                nc.vector.tensor_tensor(
                    out=ot[:, :w], in0=p[:, :w], in1=pj_part[(b, i)][:, :w],
                    op=mybir.AluOpType.add)
            else:
                nc.vector.tensor_copy(ot[:, :w], p[:, :w])
            nc.gpsimd.dma_start(
                out_d[t * 128:(t + 1) * 128, lo:lo + w], ot[:, :w])

        # software-pipelined emission
        for i in range(20):
            qkv_chain(0, i)
        qk1 = iter(range(20))
        for hp in range(6):
            attention_hp(0, hp)
            if hp % 2 == 1:
                norm_tail(0, hp // 2)
            for _ in range(4 if hp < 2 else 3):
                i = next(qk1, None)
                if i is not None:
                    qkv_chain(1, i)
        pj0 = iter(range(8))
        for hp in range(6):
            attention_hp(1, hp)
            if hp % 2 == 1:
                norm_tail(1, hp // 2)
            if hp == 3:
                for i in range(8):
                    proj_chunk(1, i, 0, 4, partial=True)
            i = next(pj0, None)
            if i is not None:
                proj_chunk(0, i)
        for i in pj0:
            proj_chunk(0, i)
        for i in range(8):
            proj_chunk(1, i, 4, KT)

    nc.compile()
    return nc


def get_compiled():
    if "nc" not in _cache:
        _cache["nc"] = _build()
    return _cache["nc"]


def make_in_maps(x, attention_mask, W_attn, b_attn, W_proj, b_proj):
    x = np.asarray(x, dtype=np.float32).astype(np.float16)
    mask = np.ascontiguousarray(
        np.asarray(attention_mask, dtype=np.float32)[:, 0, 0, :])
    wa = np.asarray(W_attn, dtype=np.float32).astype(np.float16)
    ba = np.asarray(b_attn, dtype=np.float32).astype(np.float16).reshape(1, C3)
    wp = np.asarray(W_proj, dtype=np.float32).astype(np.float16)
    bp = np.asarray(b_proj, dtype=np.float32).astype(np.float16).reshape(1, C)
    maps = []
    for i in range(N_CORES):
        maps.append({
            "x": np.ascontiguousarray(x[BC * i:BC * (i + 1)].reshape(M, C)),
            "mask": np.ascontiguousarray(mask[BC * i:BC * (i + 1)]),
            "w_attn": wa, "b_attn": ba, "w_proj": wp, "b_proj": bp,
        })
    return maps


def kernel(x, attention_mask, W_attn, b_attn, W_proj, b_proj):
    from concourse.bass_utils import run_bass_kernel_spmd

    nc = get_compiled()
    in_maps = make_in_maps(x, attention_mask, W_attn, b_attn, W_proj, b_proj)
    last_err = None
    for _ in range(3):
        try:
            res = run_bass_kernel_spmd(nc, in_maps, list(range(N_CORES)))
            break
        except Exception as e:  # transient NRT device errors: retry
            last_err = e
    else:
        raise last_err
    out = np.concatenate(
        [res.results[i]["out"].reshape(BC, T, C) for i in range(N_CORES)], axis=0)
    return out.astype(np.float32)


# revision 30
# speedup vs baseline: 1.6552x; 1.0808x over previous
"""BERT self-attention (B=16, T=512, C=768, H=12, D=64) on 8 trn2 NeuronCores.

Data-parallel over batch: each core gets 2 batches. Matmul operands are fp16
(11-bit mantissa, ~tf32-class precision, 1 cycle/row PE streaming, FWL weight
loads); all accumulation stays fp32 in PSUM. Per core:
  xT    = x transposed during load via the DMA XBAR transpose (fp16).
  Q^T/K^T ([feature, token] layout, lhsT = W_attn tile) and V ([token, feature]
          layout with an interleaved ones column per head, lhsT = xT tile).
  S^T   = K^T-as-lhsT matmul -> scores in [key, query] layout (K=64, head pairs
          packed in PE row groups via base-partition-64 slices).
  P     = exp(S/8 + mask) on ScalarE (mask is a per-partition bias in this
          layout), written as fp16.
  y^T   = lhsT=[V_h | ones] matmul -> unnormalized y^T plus softmax row-sums as
          an extra PSUM row; row-sums are collected per batch, inverted in one
          batched DVE reciprocal, replicated across partitions by a
          broadcast-AP DMA, and applied with a DVE multiply.
  out   = y^T-as-lhsT matmul with W_proj + b_proj (fp32 result to DRAM).
Biases are folded in as K=1 accumulating matmuls against a ones row.
"""

import sys

sys.path.insert(0, "/opt/trn_rl_repo")

from contextlib import ExitStack

import numpy as np

B, T, C = 16, 512, 768
H, D = 12, 64
C3 = 3 * C
N_CORES = 8
BC = B // N_CORES           # batches per core
M = BC * T                  # tokens per core
KT = C // 128               # feature k-tiles (6)
TT = M // 128               # token tiles per core (8)
NQK = 2 * C // 128          # q+k feature n-tiles (12)
VW = H * (D + 1)            # v tile width with interleaved ones cols (780)
SCALE = 1.0 / np.sqrt(D)

_cache = {}


def _build():
    import concourse.bass as bass
    import concourse.tile as tile
    from concourse import bacc, mybir
    from concourse.masks import make_identity
    f32 = mybir.dt.float32
    f16 = mybir.dt.float16
    Exp = mybir.ActivationFunctionType.Exp

    nc = bacc.Bacc("TRN2", target_bir_lowering=False, debug=False,
                   num_devices=N_CORES)
    x_d = nc.dram_tensor("x", [M, C], f16, kind="ExternalInput").ap()
    mask_d = nc.dram_tensor("mask", [BC, T], f32, kind="ExternalInput").ap()
    wa_d = nc.dram_tensor("w_attn", [C, C3], f16, kind="ExternalInput").ap()
    ba_d = nc.dram_tensor("b_attn", [1, C3], f16, kind="ExternalInput").ap()
    wp_d = nc.dram_tensor("w_proj", [C, C], f16, kind="ExternalInput").ap()
    bp_d = nc.dram_tensor("b_proj", [1, C], f16, kind="ExternalInput").ap()
    out_d = nc.dram_tensor("out", [M, C], f32, kind="ExternalOutput").ap()

    with tile.TileContext(nc) as tc, ExitStack() as ctx:
        pp = ctx.enter_context(tc.tile_pool(name="pp", bufs=1))
        np_ = ctx.enter_context(tc.tile_pool(name="norm", bufs=4))
        ap_ = ctx.enter_context(tc.tile_pool(name="att", bufs=6))
        ps_mm = ctx.enter_context(tc.tile_pool(name="ps_mm", bufs=2, space="PSUM"))

        ones = pp.tile([1, M], f16, tag="ones")
        nc.vector.memset(ones[:], 1.0)
        mask_sb = pp.tile([128, BC * 4], f32, tag="mask")
        nc.gpsimd.dma_start(
            mask_sb[:],
            mask_d.rearrange("a b -> (a b)").rearrange("(j p) -> p j", p=128))
        ba_t = pp.tile([1, C3], f16, tag="ba")
        nc.gpsimd.dma_start(ba_t[:], ba_d[:])
        bp_t = pp.tile([1, C], f16, tag="bp")
        nc.gpsimd.dma_start(bp_t[:], bp_d[:])

        wa_t = [pp.tile([128, C3], f16, tag=f"wa{k}", name=f"wa{k}")
                for k in range(KT)]
        xT = [pp.tile([128, M], f16, tag=f"xT{k}", name=f"xT{k}")
              for k in range(KT)]
        ident = pp.tile([128, 128], f16, tag="ident")
        make_identity(nc, ident[:])
        ones16 = pp.tile([128, 64], f16, tag="ones16")
        nc.vector.memset(ones16[:], 1.0)
        with tc.tile_pool(name="ps_tr", bufs=4, space="PSUM") as ps_tr, \
                tc.tile_pool(name="xin", bufs=4) as xin:
            xt_ins = []
            for t in range(TT):
                xt_in = xin.tile([128, C], f16, tag="x_in", bufs=8,
                                 name=f"x_in{t}")
                q = nc.sync if t % 2 == 0 else nc.scalar
                q.dma_start(xt_in[:], x_d[t * 128:(t + 1) * 128, :])
                xt_ins.append(xt_in)
            for j in range(2):
                for k in range(KT):
                    qw = nc.scalar if k % 2 == 0 else nc.sync
                    qw.dma_start(
                        wa_t[k][:, j * 1152:(j + 1) * 1152],
                        wa_d[k * 128:(k + 1) * 128, j * 1152:(j + 1) * 1152])
            for t in range(TT):
                for k in range(KT):
                    ptr = ps_tr.tile([128, 128], f16)
                    nc.tensor.transpose(
                        ptr[:], xt_ins[t][:, k * 128:(k + 1) * 128], ident[:])
                    nc.vector.tensor_copy(
                        xT[k][:, t * 128:(t + 1) * 128], ptr[:])
        ps_s = ctx.enter_context(tc.tile_pool(name="ps_s", bufs=2, space="PSUM"))
        ps_y = ctx.enter_context(tc.tile_pool(name="ps_y", bufs=2, space="PSUM"))
        wp_t = [pp.tile([128, C], f16, tag=f"wp{k}", name=f"wp{k}")
                for k in range(KT)]
        for k in range(KT):
            nc.gpsimd.dma_start(wp_t[k][:], wp_d[k * 128:(k + 1) * 128, :])

        v_t = [pp.tile([128, VW], f16, tag=f"v{t}", name=f"v{t}")
               for t in range(TT)]
        qkT = [pp.tile([128, M], f16, tag=f"qk{n}", name=f"qk{n}")
               for n in range(NQK)]
        yT_t = [pp.tile([128, M], f16, tag=f"yT{c}", name=f"yT{c}")
                for c in range(KT)]
        for t in range(TT):
            nc.vector.memset(
                v_t[t].rearrange("p (h c) -> p h c", c=D + 1)
                    [:, :, D:D + 1], 1.0)

        def qkv_chain(b, i):
            """i in [0, 20): 12 QK n-tiles then 8 V half-tiles."""
            bcol = b * T
            if i < NQK:
                n = i
                p = ps_mm.tile([128, 512], f32, tag="mm", name=f"mm{b}_{i}")
                for k in range(KT):
                    nc.tensor.matmul(
                        p[:],
                        wa_t[k][:, n * 128:(n + 1) * 128],
                        xT[k][:, bcol:bcol + T],
                        start=(k == 0), stop=False)
                nc.tensor.matmul(
                    p[:],
                    ba_t[0:1, n * 128:(n + 1) * 128],
                    ones[0:1, bcol:bcol + T],
                    start=False, stop=True)
                nc.vector.tensor_copy(qkT[n][:, bcol:bcol + T], p[:])
            else:
                j = i - NQK
                t = b * 4 + j // 2
                lo, w = ((0, 512), (512, 256))[j % 2]
                p = ps_mm.tile([128, 512], f32, tag="mm", name=f"mm{b}_{i}")
                for k in range(KT):
                    nc.tensor.matmul(
                        p[:, :w],
                        xT[k][:, t * 128:(t + 1) * 128],
                        wa_t[k][:, 2 * C + lo:2 * C + lo + w],
                        start=(k == 0), stop=False)
                nc.tensor.matmul(
                    p[:, :w],
                    ones[0:1, t * 128:(t + 1) * 128],
                    ba_t[0:1, 2 * C + lo:2 * C + lo + w],
                    start=False, stop=True)
                h0 = lo // D
                nc.vector.tensor_copy(
                    v_t[t].rearrange("p (h c) -> p h c", c=D + 1)
                        [:, h0:h0 + w // D, 0:D],
                    p[:, :w].rearrange("p (h c) -> p h c", c=D))

        yun_all = {}
        r_tiles = {}

        def attention_hp(b, hp):
            bcol = b * T
            if hp % 2 == 0:
                rt = np_.tile([97, 512], f32, tag="r_all", bufs=3,
                              name=f"r_all{b}_{hp // 2}")
                nc.vector.memset(rt[:], 1.0)
                r_tiles[(b, hp // 2)] = rt
            e_tiles = []
            for kt in range(4):
                ps = ps_s.tile([128, 1024], f32)
                for sub in range(2):
                    r0 = 64 * sub
                    nc.tensor.matmul(
                        ps[:, sub * 512:sub * 512 + 512],
                        qkT[6 + hp][r0:r0 + D,
                                    bcol + kt * 128:bcol + (kt + 1) * 128],
                        qkT[hp][r0:r0 + D, bcol:bcol + T],
                        start=True, stop=True)
                e = ap_.tile([128, 1024], f16, tag="e")
                nc.scalar.activation(
                    e[:], ps[:], Exp,
                    bias=mask_sb[:, b * 4 + kt:b * 4 + kt + 1],
                    scale=float(SCALE))
                e_tiles.append(e)
            for sub in range(2):
                h = 2 * hp + sub
                py = ps_y.tile([128, 512], f32)
                for kt in range(4):
                    nc.tensor.matmul(
                        py[0:D + 1, :],
                        v_t[b * 4 + kt][:, (D + 1) * h:(D + 1) * (h + 1)],
                        e_tiles[kt][:, sub * 512:sub * 512 + 512],
                        start=(kt == 0), stop=(kt == 3))
                yun = np_.tile([64, 512], f16, tag="yun", bufs=14,
                               name=f"yun{b}_{h}")
                nc.vector.tensor_copy(yun[:], py[0:D, :])
                rs = np_.tile([D + 1, 512], f32, tag="rstage")
                nc.scalar.copy(rs[D:D + 1, :], py[D:D + 1, :])
                nc.sync.dma_start(
                    r_tiles[(b, hp // 2)][32 * (h % 4):32 * (h % 4) + 1, :],
                    rs[D:D + 1, :])
                yun_all[(b, h)] = yun

        def norm_tail(b, grp):
            bcol = b * T
            recip = np_.tile([97, 512], f32, tag="recip", bufs=3)
            nc.vector.reciprocal(recip[:], r_tiles[(b, grp)][:])
            recip16 = np_.tile([97, 512], f16, tag="recip16", bufs=3)
            nc.vector.tensor_copy(recip16[:], recip[:])
            for h in range(4 * grp, 4 * grp + 4):
                nt, r0 = h // 2, 64 * (h % 2)
                j = 32 * (h % 4)
                rep = ps_y.tile([128, 512], f32, tag="py", name=f"rep{b}_{h}")
                nc.tensor.matmul(
                    rep[0:64, :], ones16[j:j + 1, :], recip16[j:j + 1, :],
                    start=True, stop=True, tile_position=(j, 0))
                dst = yT_t[nt][r0:r0 + D, bcol:bcol + T]
                if r0 == 0:
                    nc.vector.tensor_mul(dst, yun_all[(b, h)][:], rep[0:64, :])
                else:
                    st = np_.tile([64, 512], f16, tag="stage")
                    nc.vector.tensor_mul(st[:], yun_all[(b, h)][:],
                                         rep[0:64, :])
                    nc.gpsimd.dma_start(dst, st[:])

        pj_part = {}

        def proj_chunk(b, i, ks=0, ke=KT, partial=False):
            t = b * 4 + i // 2
            lo, w = ((0, 512), (512, 256))[i % 2]
            p = ps_mm.tile([128, 512], f32, tag="mm", name=f"pj{b}_{i}_{ks}")
            for k in range(ks, ke):
                nc.tensor.matmul(
                    p[:, :w],
                    yT_t[k][:, t * 128:(t + 1) * 128],
                    wp_t[k][:, lo:lo + w],
                    start=(k == ks), stop=(partial and k == ke - 1))
            if partial:
                pt = np_.tile([128, 512], f32, tag="pjpart", bufs=8,
                              name=f"pjpart{i}")
                nc.vector.tensor_copy(pt[:, :w], p[:, :w])
                pj_part[(b, i)] = pt
                return
            nc.tensor.matmul(
                p[:, :w],
                ones[0:1, t * 128:(t + 1) * 128],
                bp_t[0:1, lo:lo + w],
                start=False, stop=True)
            ot = np_.tile([128, 512], f32, tag="ostage", bufs=3)
            if (b, i) in pj_part:
                nc.vector.tensor_tensor(
                    out=ot[:, :w], in0=p[:, :w], in1=pj_part[(b, i)][:, :w],
                    op=mybir.AluOpType.add)
            else:
                nc.vector.tensor_copy(ot[:, :w], p[:, :w])
            nc.gpsimd.dma_start(
                out_d[t * 128:(t + 1) * 128, lo:lo + w], ot[:, :w])

        # software-pipelined emission
        for i in range(20):
            qkv_chain(0, i)
        qk1 = iter(range(20))
        for hp in range(6):
            attention_hp(0, hp)
            if hp % 2 == 1:
                norm_tail(0, hp // 2)
            for _ in range(4 if hp < 2 else 3):
                i = next(qk1, None)
                if i is not None:
                    qkv_chain(1, i)
        pj0 = iter(range(8))
        for hp in range(6):
            attention_hp(1, hp)
            if hp % 2 == 1:
                norm_tail(1, hp // 2)
            if hp == 3:
                for i in range(8):
                    proj_chunk(1, i, 0, 4, partial=True)
            i = next(pj0, None)
            if i is not None:
                proj_chunk(0, i)
        for i in pj0:
            proj_chunk(0, i)
        for i in range(8):
            proj_chunk(1, i, 4, KT)

    nc.compile()
    return nc


def get_compiled():
    if "nc" not in _cache:
        _cache["nc"] = _build()
    return _cache["nc"]


def make_in_maps(x, attention_mask, W_attn, b_attn, W_proj, b_proj):
    x = np.asarray(x, dtype=np.float32).astype(np.float16)
    mask = np.ascontiguousarray(
        np.asarray(attention_mask, dtype=np.float32)[:, 0, 0, :])
    wa = np.asarray(W_attn, dtype=np.float32).astype(np.float16)
    ba = np.asarray(b_attn, dtype=np.float32).astype(np.float16).reshape(1, C3)
    wp = np.asarray(W_proj, dtype=np.float32).astype(np.float16)
    bp = np.asarray(b_proj, dtype=np.float32).astype(np.float16).reshape(1, C)
    maps = []
    for i in range(N_CORES):
        maps.append({
            "x": np.ascontiguousarray(x[BC * i:BC * (i + 1)].reshape(M, C)),
            "mask": np.ascontiguousarray(mask[BC * i:BC * (i + 1)]),
            "w_attn": wa, "b_attn": ba, "w_proj": wp, "b_proj": bp,
        })
    return maps


def kernel(x, attention_mask, W_attn, b_attn, W_proj, b_proj):
    from concourse.bass_utils import run_bass_kernel_spmd

    nc = get_compiled()
    in_maps = make_in_maps(x, attention_mask, W_attn, b_attn, W_proj, b_proj)
    last_err = None
    for _ in range(3):
        try:
            res = run_bass_kernel_spmd(nc, in_maps, list(range(N_CORES)))
            break
        except Exception as e:  # transient NRT device errors: retry
            last_err = e
    else:
        raise last_err
    out = np.concatenate(
        [res.results[i]["out"].reshape(BC, T, C) for i in range(N_CORES)], axis=0)
    return out.astype(np.float32)
